# revision 1
# baseline (speedup 1.0000x reference)
"""Distributed Trainium2 kernel for a full attention block (QKV proj + RoPE +
bidirectional SDPA + output proj), SPMD across 8 NeuronCores.

Sharding: tensor-parallel over heads (16 heads -> 2 per core) for QKV+attention;
the output projection is column-sharded (each core owns 256 of the 2048 output
channels) over the AllGather'ed attention output, so no core ever needs a
rank-dependent address.

Layouts (all chosen so no on-device transposes are needed):
  - host pre-transposes x -> xT [C, B*T] and all weights -> [in, out]
  - q,k are produced directly in transposed form qT/kT [d, t] by using the
    weight as the stationary matmul operand (v in [t, d] form by swapping roles)
  - attention is computed as scoresT [tk, tq] = (kT-tile).T @ qT, softmax along
    the partition axis: exp on ACT (max-subtraction skipped: inputs are
    unit-normal so |score| <~ 6, safe in f32), denominator via a DVE running sum
    + a ones-matmul partition reduction; the division is applied after the
    attn@v matmul via a gpsimd partition-broadcast reciprocal.

dtypes: float16 for x/weights/exp/v/AG traffic (5e-4 rounding), f32/f32r for
the q,k/rope/score/softmax-denominator path (f32r matmuls run at full PE rate
for free dim >=256; measured 233ns vs 864ns plain-f32 at N=512).

Overlap structure (the engine program order is fixed at schedule time, so the
trace itself is interleaved):
  - batch-0 attention blocks (ACT-exp-bound) are traced between batch-1 QKV
    projection windows (PE-bound), so Scalar runs exp while the PE streams
    projection matmuls;
  - the AllGather is split into 4 quarter-gathers (batch x tq-half); batch-0
    projection quarters are traced between batch-1 attention blocks so the
    gathers overlap compute and only the last quarter's gather is exposed.
"""
import sys
for _p in ("/opt/trn_rl_repo",):
    if _p not in sys.path:
        sys.path.append(_p)

import numpy as np

B, T, C = 2, 2048, 2048
H, D = 16, 128
NCORES = 8
HL = H // NCORES          # heads per core = 2
TT = B * T                # 4096
NKC = C // 128            # 16 contraction chunks
TW = 512                  # t-window (psum bank width in f32)
TW2 = 1024                # wide-exp window (2 banks)
NTWB = T // TW            # 4 x-windows per batch
NTC = T // 128            # 16 tk chunks per batch
SCALE = float(1.0 / np.sqrt(D))

_CACHE = {}


def _build():
    from concourse import bacc, mybir, tile

    f32 = mybir.dt.float32
    f32r = mybir.dt.float32r
    f16 = mybir.dt.float16
    EXP = mybir.ActivationFunctionType.Exp

    nc = bacc.Bacc("TRN2", target_bir_lowering=False, debug=False,
                   num_devices=NCORES)

    xT_ext = nc.dram_tensor("xT", [C, TT], f16, kind="ExternalInput")
    wqk_ext = nc.dram_tensor("wqkT", [C, 4 * 128], f16, kind="ExternalInput")
    wv_ext = nc.dram_tensor("wvT", [C, HL * 128], f16, kind="ExternalInput")
    wp_ext = nc.dram_tensor("wpT", [C, 256], f16, kind="ExternalInput")
    cos_ext = nc.dram_tensor("cosT", [128, TT], f16, kind="ExternalInput")
    sin_ext = nc.dram_tensor("sinTs", [128, TT], f16, kind="ExternalInput")
    out_ext = nc.dram_tensor("outT", [256, TT], f32, kind="ExternalOutput")

    with tile.TileContext(nc) as tc:
        with tc.tile_pool(name="dram", bufs=1, space="DRAM") as dram:
            # f32 spill for rope'd q,k, per batch: mi in {q_h0,q_h1,k_h0,k_h1}
            qk_dram = [dram.tile([4, 128, T], f16, tag=f"qkd{b}",
                                 name=f"qkd{b}") for b in range(B)]
            y_dram = [[dram.tile([HL * 128, TW2], f16, tag=f"yd{b}{hf}",
                                 name=f"yd{b}{hf}") for hf in range(2)]
                      for b in range(B)]
            ag_dram = [[dram.tile([H * 128, TW2], f16, tag=f"agd{b}{hf}",
                                  name=f"agd{b}{hf}", addr_space="Shared")
                        for hf in range(2)] for b in range(B)]

            with (
                # one PSUM pool, 3 tags, 8 banks total:
                #   mmA: 2-bank slots x2 (qk-proj accum, wide scores)
                #   mmB: 1-bank x2 (v-proj, attn@v, proj accum)
                #   sr:  1-bank x2 (colsum [1,TW])
                tc.tile_pool(name="psum", bufs=2, space="PSUM") as psum,
                tc.tile_pool(name="pV", bufs=1) as pV,
            ):
                v_sb = pV.tile([128, TT // 128, HL * 128], f16, tag="v")

                # Pool stack (LIFO close order): pB [attention, whole kernel],
                # pA [x/w slabs, through phase A], pR [rope scratch+tables,
                # phase A only]. pR and pA close before pC (projection) opens.
                pB_cm = tc.tile_pool(name="pB", bufs=1)
                pB = pB_cm.__enter__()
                pA_cm = tc.tile_pool(name="pA", bufs=1)
                pA = pA_cm.__enter__()
                pR_cm = tc.tile_pool(name="pR", bufs=1)
                pR = pR_cm.__enter__()

                # ---- phase A prologue -------------------------------------
                wqk_sb = pA.tile([128, NKC, 4 * 128], f16, tag="wqk")
                for hchunk in range(2):
                    nc.sync.dma_start(
                        wqk_sb[:, hchunk * 8:(hchunk + 1) * 8, :],
                        wqk_ext[hchunk * 8 * 128:(hchunk + 1) * 8 * 128, :]
                        .rearrange("(kc p) o -> p kc o", p=128))
                wv_sb = pA.tile([128, NKC, HL * 128], f16, tag="wv")
                nc.sync.dma_start(
                    wv_sb[:],
                    wv_ext[:].rearrange("(kc p) o -> p kc o", p=128))
                cos_sb = pR.tile([128, TT], f16, tag="cos")
                sin_sb = pR.tile([128, TT], f16, tag="sin")

                def phase_a_window(b, twb):
                    """QKV projection + rope for one 512-wide t window."""
                    tw = b * NTWB + twb
                    x_sb = pA.tile([128, NKC, TW], f16, tag="x", bufs=2,
                                   name="x_sb")
                    if tw == 0:
                        for hchunk in range(2):
                            nc.sync.dma_start(
                                x_sb[:, hchunk * 8:(hchunk + 1) * 8, :],
                                xT_ext[hchunk * 8 * 128:(hchunk + 1) * 8 * 128,
                                       tw * TW:(tw + 1) * TW]
                                .rearrange("(kc p) t -> p kc t", p=128))
                    else:
                        for q4 in range(4):
                            nc.sync.dma_start(
                                x_sb[:, q4 * 4:(q4 + 1) * 4, :],
                                xT_ext[q4 * 4 * 128:(q4 + 1) * 4 * 128,
                                       tw * TW:(tw + 1) * TW]
                                .rearrange("(kc p) t -> p kc t", p=128))
                    cs = slice(tw * TW, (tw + 1) * TW)
                    csb = slice(twb * TW, (twb + 1) * TW)
                    for mi in range(4):
                        pqk = psum.tile([128, TW], f32, tag="sr",
                                        name="pqk")
                        for kc in range(NKC):
                            nc.tensor.matmul(
                                pqk[:],
                                wqk_sb[:, kc, mi * 128:(mi + 1) * 128],
                                x_sb[:, kc, :],
                                start=(kc == 0), stop=(kc == NKC - 1))
                        if tw == 0 and mi == 0:
                            nc.sync.dma_start(cos_sb[:], cos_ext[:])
                            nc.sync.dma_start(sin_sb[:], sin_ext[:])
                        # RoPE: q' = q*cos + swap_halves(q)*sin_signed
                        qraw = pR.tile([128, TW], f32, tag="qraw", bufs=2,
                                       name="qraw")
                        nc.scalar.copy(qraw[:], pqk[:])
                        qrot = pR.tile([128, TW], f32, tag="qrot", bufs=2,
                                       name="qrot")
                        nc.sync.dma_start(qrot[0:64, :], qraw[64:128, :])
                        nc.sync.dma_start(qrot[64:128, :], qraw[0:64, :])
                        qfin = pR.tile([128, TW], f16, tag="qfin", bufs=2,
                                       name="qfin")
                        nc.vector.tensor_mul(qfin[:], qraw[:], cos_sb[:, cs])
                        nc.vector.tensor_mul(qrot[:], qrot[:], sin_sb[:, cs])
                        nc.vector.tensor_add(qfin[:], qfin[:], qrot[:])
                        nc.sync.dma_start(qk_dram[b][mi, :, csb], qfin[:])
                    for tci in range(TW // 128):
                        tc_g = tw * (TW // 128) + tci
                        pv = psum.tile([128, HL * 128], f32, tag="mmB",
                                       name="pv")
                        for kc in range(NKC):
                            nc.tensor.matmul(
                                pv[:],
                                x_sb[:, kc, tci * 128:(tci + 1) * 128],
                                wv_sb[:, kc, :],
                                start=(kc == 0), stop=(kc == NKC - 1))
                        nc.vector.tensor_copy(v_sb[:, tc_g, :], pv[:])

                # ---- attention helpers ------------------------------------
                ones32 = pB.tile([128, 1], f32, tag="ones32")
                nc.vector.memset(ones32[:], 1.0)
                ones_r = pB.tile([128, 1], f32r, tag="onesr")
                nc.vector.tensor_copy(ones_r[:], ones32[:])

                def load_qk(b):
                    qk_t = []
                    for h in range(HL):
                        qh = pB.tile([128, T], f16, tag=f"qh{h}", bufs=1,
                                     name=f"qh{h}")
                        nc.sync.dma_start(qh[:], qk_dram[b][h])
                        kh = pB.tile([128, T], f16, tag=f"kh{h}", bufs=1,
                                     name=f"kh{h}")
                        nc.sync.dma_start(kh[:], qk_dram[b][2 + h])
                        qk_t.append((qh, kh))
                    return qk_t

                def attn_block(b, hf, h, qk_t):
                    """scoresT+softmax+attn@v for one (batch, tq-half, head)."""
                    qh, kh = qk_t[h]
                    exp_tiles = []
                    ssum = pB.tile([128, TW2], f32r, tag="ssum", bufs=1,
                                   name="ssum")
                    for tkc in range(NTC):
                        sc = psum.tile([128, TW2], f32, tag="mmA", name="sc")
                        for j in range(2):
                            tq0 = hf * TW2 + j * TW
                            nc.tensor.matmul(
                                sc[:, j * TW:(j + 1) * TW],
                                kh[:, tkc * 128:(tkc + 1) * 128],
                                qh[:, tq0:tq0 + TW],
                                start=True, stop=True)
                        e = pB.tile([128, TW2], f16, tag=f"e{tkc}",
                                    bufs=2, name=f"e{tkc}")
                        nc.scalar.activation(e[:], sc[:], EXP, scale=SCALE)
                        exp_tiles.append(e)
                        if tkc == 0:
                            nc.vector.tensor_copy(ssum[:], e[:])
                        else:
                            nc.vector.tensor_add(ssum[:],
                                                 ssum[:].bitcast(f32), e[:])
                    for j in range(2):
                        py = psum.tile([128, TW], f32, tag="mmB", name="py")
                        for tkc in range(NTC):
                            nc.tensor.matmul(
                                py[:],
                                v_sb[:, b * NTC + tkc, h * 128:(h + 1) * 128],
                                exp_tiles[tkc][:, j * TW:(j + 1) * TW],
                                start=(tkc == 0), stop=(tkc == NTC - 1))
                        ps1 = psum.tile([1, TW], f32, tag="sr", name="ps1")
                        nc.tensor.matmul(ps1[:], ones_r[:],
                                         ssum[:, j * TW:(j + 1) * TW],
                                         start=True, stop=True)
                        recip = pB.tile([1, TW], f32, tag="recip", bufs=2,
                                        name="recip")
                        nc.vector.reciprocal(recip[:], ps1[:])
                        rbs = pB.tile([128, TW], f32, tag="rbs", bufs=2,
                                      name="rbs")
                        nc.gpsimd.partition_broadcast(rbs[:], recip[:])
                        ybf = pB.tile([128, TW], f16, tag="ybf", bufs=2,
                                      name="ybf")
                        nc.vector.tensor_mul(ybf[:], py[:], rbs[:])
                        nc.sync.dma_start(
                            y_dram[b][hf][h * 128:(h + 1) * 128,
                                          j * TW:(j + 1) * TW],
                            ybf[:])

                def all_gather(b, hf):
                    nc.gpsimd.collective_compute(
                        "AllGather",
                        mybir.AluOpType.bypass,
                        replica_groups=[list(range(NCORES))],
                        ins=[y_dram[b][hf][:]],
                        outs=[ag_dram[b][hf][:]],
                    )

                # ---- trace schedule ---------------------------------------
                # phase A batch 0 alone (attention has nothing to do yet)
                for twb in range(NTWB):
                    phase_a_window(0, twb)
                # batch-0 attention interleaved with batch-1 phase A windows
                qk0 = load_qk(0)
                blocks0 = [(hf, h) for hf in range(2) for h in range(HL)]
                for i, twb in enumerate(range(NTWB)):
                    phase_a_window(1, twb)
                    hf, h = blocks0[i]
                    attn_block(0, hf, h, qk0)
                    if h == HL - 1:
                        all_gather(0, hf)
                qk1 = load_qk(1)

                # phase A scratch + slabs are dead now
                pR_cm.__exit__(None, None, None)
                pA_cm.__exit__(None, None, None)

                # batch-1 attention with batch-0 projection woven between
                with tc.tile_pool(name="pC", bufs=1) as pC:
                    wp_sb = pC.tile([128, NKC, 256], f16, tag="wp")
                    nc.sync.dma_start(
                        wp_sb[:],
                        wp_ext[:].rearrange("(kc p) o -> p kc o", p=128))

                    def proj_strip(b, hf, j):
                        ag_sb = pC.tile([128, NKC, TW], f16, tag="ag",
                                        bufs=2, name="ag_sb")
                        nc.sync.dma_start(
                            ag_sb[:],
                            ag_dram[b][hf][:, j * TW:(j + 1) * TW]
                            .rearrange("(kc p) t -> p kc t", p=128))
                        for coc in range(2):
                            po = psum.tile([128, TW], f32, tag="sr",
                                           name="po")
                            for kc in range(NKC):
                                nc.tensor.matmul(
                                    po[:],
                                    wp_sb[:, kc, coc * 128:(coc + 1) * 128],
                                    ag_sb[:, kc, :],
                                    start=(kc == 0), stop=(kc == NKC - 1))
                            od = pC.tile([128, TW], f32, tag="od", bufs=2,
                                         name="od")
                            nc.vector.tensor_copy(od[:], po[:])
                            t0 = b * T + hf * TW2 + j * TW
                            nc.sync.dma_start(
                                out_ext[coc * 128:(coc + 1) * 128,
                                        t0:t0 + TW],
                                od[:])

                    attn_block(1, 0, 0, qk1)
                    attn_block(1, 0, 1, qk1)
                    all_gather(1, 0)
                    proj_strip(0, 0, 0)
                    attn_block(1, 1, 0, qk1)
                    proj_strip(0, 0, 1)
                    attn_block(1, 1, 1, qk1)
                    all_gather(1, 1)
                    proj_strip(0, 1, 0)
                    proj_strip(0, 1, 1)
                    proj_strip(1, 0, 0)
                    proj_strip(1, 0, 1)
                    proj_strip(1, 1, 0)
                    proj_strip(1, 1, 1)

                pB_cm.__exit__(None, None, None)
    nc.compile()
    return nc


def _prepare_in_maps(x, cos, sin, Wqkv, Wproj):
    f16 = np.float16
    xT = np.ascontiguousarray(x.reshape(TT, C).T).astype(f16)
    cosT = np.ascontiguousarray(np.tile(cos.T, (1, B))).astype(f16)
    sinS = sin.T.astype(np.float32).copy()
    sinS[:D // 2] *= -1.0
    sinTs = np.ascontiguousarray(np.tile(sinS, (1, B))).astype(f16)
    Wq, Wk, Wv = Wqkv[0:C], Wqkv[C:2 * C], Wqkv[2 * C:3 * C]

    in_maps = []
    for c in range(NCORES):
        hs = [HL * c + j for j in range(HL)]
        wqk_rows = np.concatenate(
            [Wq[h * D:(h + 1) * D] for h in hs]
            + [Wk[h * D:(h + 1) * D] for h in hs], axis=0)
        wv_rows = np.concatenate([Wv[h * D:(h + 1) * D] for h in hs], axis=0)
        in_maps.append({
            "xT": xT,
            "wqkT": np.ascontiguousarray(wqk_rows.T).astype(f16),
            "wvT": np.ascontiguousarray(wv_rows.T).astype(f16),
            "wpT": np.ascontiguousarray(
                Wproj[c * 256:(c + 1) * 256, :].T).astype(f16),
            "cosT": cosT,
            "sinTs": sinTs,
        })
    return in_maps


def run_sharded(x, cos, sin, Wqkv, Wproj, trace=False):
    """Compile (cached), run on 8 cores, return (out, BassKernelResults)."""
    from concourse.bass_utils import run_bass_kernel_spmd

    if "nc" not in _CACHE:
        _CACHE["nc"] = _build()
    nc = _CACHE["nc"]
    in_maps = _prepare_in_maps(x, cos, sin, Wqkv, Wproj)
    res = run_bass_kernel_spmd(nc, in_maps, core_ids=list(range(NCORES)),
                               trace=trace)
    out = np.empty((B, T, C), dtype=np.float32)
    for c in range(NCORES):
        outT = res.results[c]["outT"]          # [256, TT]
        out[:, :, c * 256:(c + 1) * 256] = \
            outT.reshape(256, B, T).transpose(1, 2, 0)
    return out, res


def kernel(x, cos, sin, Wqkv, Wproj):
    out, _ = run_sharded(x, cos, sin, Wqkv, Wproj, trace=False)
    return out



# revision 3
# speedup vs baseline: 1.1479x; 1.1479x over previous
"""Distributed Trainium2 kernel for a full attention block (QKV proj + RoPE +
bidirectional SDPA + output proj), SPMD across 8 NeuronCores.

Sharding: tensor-parallel over heads (16 heads -> 2 per core) for QKV+attention;
the output projection is column-sharded (each core owns 256 of the 2048 output
channels) over AllGather'ed attention outputs.

v2 rewrite, driven by trace analysis of the v1 kernel (539-584us):
  - the PE streams matmul columns at ~0.506 ns/col regardless of dtype (f16
    already full rate), so the per-core floor is ~794k cols ~= 402us; all the
    headroom is PE idle time, not matmul rate.
  - fine-grained software pipelining: each attention block is emitted as 16
    (scores -> exp -> ssum -> attn@v[skew-2]) units with QKV-window / output-
    proj matmul streams woven between them, so the in-order PE queue never
    waits on the Scalar-engine exp chain (v1 lost ~90us there).
  - q,k stay SBUF-resident (v1 spilled to DRAM and reloaded, 8MB + a 22us
    stall before batch-1 attention).
  - host pre-chunks x/weights into the exact SBUF layout so window loads are
    128 descriptors x 16KB instead of 2048 x 1KB; the first-window load is
    split so the first matmul starts ~4us in (v1: 26us).
  - softmax denominator: f16 SBUF-only DVE accumulation (2x/4x DVE modes),
    partition-reduction via a ones-matmul into a rotating psum slot,
    reciprocal_approx_fast + gpsimd partition_broadcast (v1 burned 4us per
    block in a single-lane f32 reciprocal).
  - the AllGather is split per head (8 x 2MB instead of 4 x 4MB) and fired
    immediately after each block; proj strips are ordered so only a few us of
    the last gather are exposed.

dtypes: f16 for x/weights/q/k/exp/v/AG traffic and the f16 ssum partials
(error budget analysed: denominator rel err ~2e-4), f32 psum accumulation
everywhere, f16 output (host converts to f32).
"""
import sys
for _p in ("/opt/trn_rl_repo",):
    if _p not in sys.path:
        sys.path.append(_p)

import numpy as np

B, T, C = 2, 2048, 2048
H, D = 16, 128
NCORES = 8
HL = H // NCORES          # heads per core = 2
TT = B * T                # 4096
NKC = C // 128            # 16 contraction chunks
TW = 512                  # t-window (psum bank width in f32)
TW2 = 1024                # block tq width (2 banks)
NTWB = T // TW            # 4 x-windows per batch
NW = B * NTWB             # 8 windows total
NTC = T // 128            # 16 tk chunks per batch
SCALE = float(1.0 / np.sqrt(D))

_CACHE = {}


def _build():
    from concourse import bacc, mybir, tile

    f32 = mybir.dt.float32
    f16 = mybir.dt.float16
    EXP = mybir.ActivationFunctionType.Exp

    nc = bacc.Bacc("TRN2", target_bir_lowering=False, debug=False,
                   num_devices=NCORES)

    # host pre-chunked layouts: per-partition-contiguous, no DMA rearranges
    xPC = nc.dram_tensor("xPC", [NW, 128, NKC * TW], f16, kind="ExternalInput")
    wqkPC = nc.dram_tensor("wqkPC", [128, NKC * 512], f16, kind="ExternalInput")
    wvPC = nc.dram_tensor("wvPC", [128, NKC * 256], f16, kind="ExternalInput")
    wpPC = nc.dram_tensor("wpPC", [128, NKC * 256], f16, kind="ExternalInput")
    cosPC = nc.dram_tensor("cosPC", [128, T], f16, kind="ExternalInput")
    sinPC = nc.dram_tensor("sinPC", [128, T], f16, kind="ExternalInput")
    outT = nc.dram_tensor("outT", [256, TT], f16, kind="ExternalOutput")

    with tile.TileContext(nc) as tc:
        with tc.tile_pool(name="dram", bufs=1, space="DRAM") as dram:
            # per (batch, tq-half, local head): y written by attention, ag is
            # the 8-core gather (global heads == h mod 2, core-major)
            y_d = [[[dram.tile([128, TW2], f16, tag=f"y{b}{hf}{h}",
                               name=f"y{b}{hf}{h}") for h in range(HL)]
                    for hf in range(2)] for b in range(B)]
            ag_d = [[[dram.tile([NCORES * 128, TW2], f16, tag=f"ag{b}{hf}{h}",
                                name=f"ag{b}{hf}{h}", addr_space="Shared")
                      for h in range(HL)] for hf in range(2)] for b in range(B)]

            with (
                # PSUM tags (8 banks): sc [128,1024]x2 (scores + denom slots),
                # py [128,1024]x1 (attn@v accum), w [128,512]x2 (qk-proj,
                # v-proj, out-proj streams)
                tc.tile_pool(name="psum", bufs=2, space="PSUM") as psum,
                tc.tile_pool(name="pB", bufs=1) as pB,
            ):
                v_sb = pB.tile([128, B * NTC, HL * 128], f16, tag="v")
                # q_h0, q_h1, k_h0, k_h1 per batch, SBUF-resident
                qk_sb = [[pB.tile([128, T], f16, tag=f"qk{b}{mi}",
                                  name=f"qk{b}{mi}") for mi in range(4)]
                         for b in range(B)]
                ones16 = pB.tile([128, 1], f16, tag="ones16")
                nc.vector.memset(ones16[:], 1.0)

                pA_cm = tc.tile_pool(name="pA", bufs=1)
                pA = pA_cm.__enter__()
                pR_cm = tc.tile_pool(name="pR", bufs=1)
                pR = pR_cm.__enter__()

                # ---- phase A prologue: critical-path-ordered split DMAs ----
                wqk4 = [pA.tile([128, 4, 512], f16, tag=f"wqk{g}",
                                name=f"wqk{g}") for g in range(4)]
                x0g = [pA.tile([128, 4, TW], f16, tag=f"x0{g}",
                               name=f"x0{g}") for g in range(4)]
                cos_sb = pA.tile([128, T], f16, tag="cos")
                sin_sb = pA.tile([128, T], f16, tag="sin")
                wv_sb = pA.tile([128, NKC, 256], f16, tag="wv")

                nc.sync.dma_start(wqk4[0][:],
                                  wqkPC[:, 0:4 * 512]
                                  .rearrange("p (kc o) -> p kc o", kc=4))
                nc.sync.dma_start(x0g[0][:],
                                  xPC[0, :, 0:4 * TW]
                                  .rearrange("p (kc t) -> p kc t", kc=4))
                nc.sync.dma_start(cos_sb[:], cosPC[:])
                nc.sync.dma_start(sin_sb[:], sinPC[:])
                for g in range(1, 4):
                    nc.sync.dma_start(wqk4[g][:],
                                      wqkPC[:, g * 4 * 512:(g + 1) * 4 * 512]
                                      .rearrange("p (kc o) -> p kc o", kc=4))
                    nc.sync.dma_start(x0g[g][:],
                                      xPC[0, :, g * 4 * TW:(g + 1) * 4 * TW]
                                      .rearrange("p (kc t) -> p kc t", kc=4))
                nc.sync.dma_start(wv_sb[:],
                                  wvPC[:].rearrange("p (kc o) -> p kc o",
                                                    kc=NKC))

                xtiles = {}

                def xacc(tw):
                    if tw == 0:
                        return lambda kc: x0g[kc // 4][:, kc % 4, :]
                    t = xtiles[tw]
                    return lambda kc: t[:, kc, :]

                def prefetch_x(tw):
                    t = pA.tile([128, NKC, TW], f16, tag="x", bufs=2,
                                name="x_sb")
                    nc.sync.dma_start(
                        t[:], xPC[tw].rearrange("p (kc t) -> p kc t", kc=NKC))
                    xtiles[tw] = t

                def win_gen(tw):
                    """QKV projection + rope for one 512-wide t window.
                    8 yields: 4 q/k mi-streams, 4 v tci-streams."""
                    b, twb = divmod(tw, NTWB)
                    xat = xacc(tw)
                    cs = slice(twb * TW, (twb + 1) * TW)
                    for mi in range(4):
                        pqk = psum.tile([128, TW], f32, tag="w", name="pqk")
                        for kc in range(NKC):
                            nc.tensor.matmul(
                                pqk[:],
                                wqk4[kc // 4][:, kc % 4,
                                              mi * 128:(mi + 1) * 128],
                                xat(kc),
                                start=(kc == 0), stop=(kc == NKC - 1))
                        # RoPE: q' = q*cos + swap_halves(q)*sin_signed
                        qraw = pR.tile([128, TW], f16, tag="qraw", bufs=2,
                                       name="qraw")
                        nc.scalar.copy(qraw[:], pqk[:])
                        qrot = pR.tile([128, TW], f16, tag="qrot", bufs=2,
                                       name="qrot")
                        nc.sync.dma_start(qrot[0:64, :], qraw[64:128, :])
                        nc.sync.dma_start(qrot[64:128, :], qraw[0:64, :])
                        dst = qk_sb[b][mi][:, cs]
                        nc.vector.tensor_mul(dst, qraw[:], cos_sb[:, cs])
                        nc.vector.tensor_mul(qrot[:], qrot[:], sin_sb[:, cs])
                        nc.vector.tensor_add(dst, dst, qrot[:])
                        if mi == 0 and tw + 1 < NW:
                            prefetch_x(tw + 1)
                        yield
                    for tci in range(4):
                        pv = psum.tile([128, TW], f32, tag="w", name="pv")
                        for kc in range(NKC):
                            nc.tensor.matmul(
                                pv[:, 0:256],
                                xat(kc)[:, tci * 128:(tci + 1) * 128],
                                wv_sb[:, kc, :],
                                start=(kc == 0), stop=(kc == NKC - 1))
                        nc.vector.tensor_copy(v_sb[:, tw * 4 + tci, :],
                                              pv[:, 0:256])
                        yield

                def attn_gen(b, hf, h):
                    """scoresT+softmax+attn@v for one (batch, tq-half, head).
                    17 yields: 16 pipelined tkc units + tail."""
                    qh, kh = qk_sb[b][h], qk_sb[b][2 + h]
                    ssum = pB.tile([128, TW2], f16, tag="ssum", bufs=2,
                                   name="ssum")
                    py = psum.tile([128, TW2], f32, tag="py", bufs=1,
                                   name="py")
                    es = []

                    def attnv(i):
                        for j in range(2):
                            nc.tensor.matmul(
                                py[:, j * TW:(j + 1) * TW],
                                v_sb[:, b * NTC + i, h * 128:(h + 1) * 128],
                                es[i][:, j * TW:(j + 1) * TW],
                                start=(i == 0), stop=(i == NTC - 1))

                    for tkc in range(NTC):
                        sc = psum.tile([128, TW2], f32, tag="sc", name="sc")
                        for j in range(2):
                            tq0 = hf * TW2 + j * TW
                            nc.tensor.matmul(
                                sc[:, j * TW:(j + 1) * TW],
                                kh[:, tkc * 128:(tkc + 1) * 128],
                                qh[:, tq0:tq0 + TW],
                                start=True, stop=True)
                        e = pB.tile([128, TW2], f16, tag="e", bufs=4,
                                    name="e")
                        es.append(e)
                        nc.scalar.activation(e[:], sc[:], EXP, scale=SCALE)
                        if tkc == 0:
                            nc.vector.tensor_copy(ssum[:], e[:])
                        else:
                            nc.vector.tensor_add(ssum[:], ssum[:], e[:])
                        if tkc >= 2:
                            attnv(tkc - 2)
                        yield
                    attnv(NTC - 2)
                    attnv(NTC - 1)
                    # denominator: partition-reduce ssum via ones-matmul into
                    # a rotating sc slot, then approx-reciprocal + broadcast
                    dn = psum.tile([128, TW2], f32, tag="sc", name="dn")
                    for j in range(2):
                        nc.tensor.matmul(dn[0:1, j * TW:(j + 1) * TW],
                                         ones16[:],
                                         ssum[:, j * TW:(j + 1) * TW],
                                         start=True, stop=True)
                    rc = pB.tile([1, TW2], f32, tag="rc", bufs=2, name="rc")
                    nc.vector.reciprocal_approx_fast(rc[:], dn[0:1, :])
                    rbs = pB.tile([128, TW2], f32, tag="rbs", bufs=2,
                                  name="rbs")
                    nc.gpsimd.partition_broadcast(rbs[:], rc[:])
                    ybf = pB.tile([128, TW2], f16, tag="ybf", bufs=2,
                                  name="ybf")
                    for j in range(2):
                        nc.vector.tensor_mul(ybf[:, j * TW:(j + 1) * TW],
                                             py[:, j * TW:(j + 1) * TW],
                                             rbs[:, j * TW:(j + 1) * TW])
                    nc.sync.dma_start(y_d[b][hf][h][:], ybf[:])
                    nc.gpsimd.collective_compute(
                        "AllGather",
                        mybir.AluOpType.bypass,
                        replica_groups=[list(range(NCORES))],
                        ins=[y_d[b][hf][h][:]],
                        outs=[ag_d[b][hf][h][:]],
                    )
                    yield

                def drive(gen, n=None):
                    if n is None:
                        for _ in gen:
                            pass
                    else:
                        for _ in range(n):
                            next(gen)

                # ---- front half: batch-0 windows, then batch-1 windows
                # interleaved with batch-0 attention ----------------------
                for twb in range(NTWB):
                    drive(win_gen(twb))
                for i in range(NTWB):
                    w = win_gen(NTWB + i)
                    a = attn_gen(0, i // 2, i % 2)
                    for _ in range(8):
                        drive(w, 1)
                        drive(a, 2)
                    drive(a)

                pR_cm.__exit__(None, None, None)
                pA_cm.__exit__(None, None, None)

                # ---- back half: batch-1 attention with batch-0 proj woven,
                # then the remaining proj strips ---------------------------
                with tc.tile_pool(name="pC", bufs=1) as pC:
                    wp_sb = pC.tile([128, NKC, 256], f16, tag="wp")
                    nc.sync.dma_start(
                        wp_sb[:],
                        wpPC[:].rearrange("p (kc o) -> p kc o", kc=NKC))

                    ag_sb = {}

                    def load_ag(b, hf):
                        tl = []
                        for h in range(HL):
                            t = pC.tile([128, NKC // 2, TW2], f16,
                                        tag=f"ag{h}", bufs=2, name=f"ag{h}")
                            nc.sync.dma_start(
                                t[:],
                                ag_d[b][hf][h][:]
                                .rearrange("(kc p) t -> p kc t", p=128))
                            tl.append(t)
                        ag_sb[(b, hf)] = tl

                    def proj_gen(b, hf, j):
                        """output proj for one 512-t strip; 2 yields."""
                        agt = ag_sb[(b, hf)]
                        for coc in range(2):
                            po = psum.tile([128, TW], f32, tag="w",
                                           name="po")
                            for kc in range(NKC):
                                par, kcl = divmod(kc, NKC // 2)
                                nc.tensor.matmul(
                                    po[:],
                                    wp_sb[:, kc, coc * 128:(coc + 1) * 128],
                                    agt[par][:, kcl, j * TW:(j + 1) * TW],
                                    start=(kc == 0), stop=(kc == NKC - 1))
                            od = pC.tile([128, TW], f16, tag="od", bufs=2,
                                         name="od")
                            nc.vector.tensor_copy(od[:], po[:])
                            t0 = b * T + hf * TW2 + j * TW
                            nc.sync.dma_start(
                                outT[coc * 128:(coc + 1) * 128, t0:t0 + TW],
                                od[:])
                            yield

                    load_ag(0, 0)
                    load_ag(0, 1)
                    strips = [(0, 0, 0), (0, 0, 1), (0, 1, 0), (0, 1, 1)]
                    for i in range(4):
                        a = attn_gen(1, i // 2, i % 2)
                        p = proj_gen(*strips[i])
                        for u in range(NTC):
                            drive(a, 1)
                            if u in (5, 11):
                                drive(p, 1)
                        drive(a)
                        drive(p)
                        if i == 1:
                            load_ag(1, 0)
                        elif i == 3:
                            load_ag(1, 1)
                    for s in [(1, 0, 0), (1, 0, 1), (1, 1, 0), (1, 1, 1)]:
                        drive(proj_gen(*s))
    nc.compile()
    return nc


def _prepare_in_maps(x, cos, sin, Wqkv, Wproj):
    f16 = np.float16

    def chunk(a):
        # [NKC*128, N] -> [128, NKC*N] per-partition-contiguous kc-major
        n = a.shape[1]
        return np.ascontiguousarray(
            a.reshape(NKC, 128, n).transpose(1, 0, 2).reshape(128, NKC * n))

    xT = x.reshape(TT, C).T.astype(f16)                      # [C, TT]
    xPC = np.empty((NW, 128, NKC * TW), dtype=f16)
    for tw in range(NW):
        xPC[tw] = chunk(xT[:, tw * TW:(tw + 1) * TW])
    cosPC = np.ascontiguousarray(cos.T).astype(f16)          # [128, T]
    sinS = sin.T.astype(np.float32).copy()
    sinS[:D // 2] *= -1.0
    sinPC = np.ascontiguousarray(sinS).astype(f16)
    Wq, Wk, Wv = Wqkv[0:C], Wqkv[C:2 * C], Wqkv[2 * C:3 * C]

    # proj contraction permutation: AllGather is per local head, so the
    # gathered y channels are [even global heads (1024), odd (1024)]
    perm = ([g * D + d for g in range(0, H, 2) for d in range(D)]
            + [g * D + d for g in range(1, H, 2) for d in range(D)])

    in_maps = []
    for c in range(NCORES):
        hs = [HL * c + j for j in range(HL)]
        wqk_rows = np.concatenate(
            [Wq[h * D:(h + 1) * D] for h in hs]
            + [Wk[h * D:(h + 1) * D] for h in hs], axis=0)
        wv_rows = np.concatenate([Wv[h * D:(h + 1) * D] for h in hs], axis=0)
        wp_rows = Wproj[c * 256:(c + 1) * 256, :].T[perm, :]
        in_maps.append({
            "xPC": xPC,
            "wqkPC": chunk(wqk_rows.T.astype(f16)),
            "wvPC": chunk(wv_rows.T.astype(f16)),
            "wpPC": chunk(wp_rows.astype(f16)),
            "cosPC": cosPC,
            "sinPC": sinPC,
        })
    return in_maps


def run_sharded(x, cos, sin, Wqkv, Wproj, trace=False):
    """Compile (cached), run on 8 cores, return (out, BassKernelResults)."""
    from concourse.bass_utils import run_bass_kernel_spmd

    if "nc" not in _CACHE:
        _CACHE["nc"] = _build()
    nc = _CACHE["nc"]
    in_maps = _prepare_in_maps(np.asarray(x), np.asarray(cos),
                               np.asarray(sin), np.asarray(Wqkv),
                               np.asarray(Wproj))
    res = run_bass_kernel_spmd(nc, in_maps, core_ids=list(range(NCORES)),
                               trace=trace)
    out = np.empty((B, T, C), dtype=np.float32)
    for c in range(NCORES):
        oT = res.results[c]["outT"].astype(np.float32)     # [256, TT]
        out[:, :, c * 256:(c + 1) * 256] = \
            oT.reshape(256, B, T).transpose(1, 2, 0)
    return out, res


def kernel(x, cos, sin, Wqkv, Wproj):
    out, _ = run_sharded(x, cos, sin, Wqkv, Wproj, trace=False)
    return out


# revision 12
# speedup vs baseline: 1.2434x; 1.0831x over previous
"""Distributed Trainium2 kernel for a full attention block (QKV proj + RoPE +
bidirectional SDPA + output proj), SPMD across 8 NeuronCores.

Sharding: tensor-parallel over heads (16 heads -> 2 per core) for QKV+attention.
The merge for the output projection uses 4 AllToAlls (one per batch x tq-half
quarter): each core keeps a 128-token slice of every quarter and computes ALL
2048 output channels for its 512 owned tokens. A2A moves each y element once
(~0.45MB wire/core per quarter) vs AllGather's 8x (every element to every
core), which un-bottlenecks the serial CC stream that dominated the v2 tail.

v3 structure (v2 measured 509us, v1 539-584us; PE floor ~402us at the
observed ~0.5ns/col stream rate):
  - front half: 4 batch-0 QKV/RoPE windows (window 0 is kc-outer so the first
    matmul starts ~3us in, streaming against the split weight/x DMAs), then 4
    slots of batch-1 window || batch-0 attention block, with the block's 16
    (scores->exp->ssum->attn@v[skew-2]) units woven between the window's 8
    matmul streams so the in-order PE never waits on the Scalar-engine exp.
  - A2A(0,hf) fires mid-front; A2A(1,hf) fires as back-half blocks finish.
  - back half: 4 batch-1 blocks with batch-0 proj quarters woven between
    units; batch-1 proj runs last, its A2A wait hidden under the (1,0) proj.
  - q,k SBUF-resident; softmax denominator = f16 SBUF DVE accumulation +
    ones-matmul partition reduction + reciprocal_approx_fast + gpsimd
    broadcast.

dtypes: f16 everywhere on the matmul/exp/collective path, f32 psum, f16
output (host converts); full Wproj (8MB f16) is SBUF-resident per core.
"""
import sys
for _p in ("/opt/trn_rl_repo",):
    if _p not in sys.path:
        sys.path.append(_p)

import numpy as np

B, T, C = 2, 2048, 2048
H, D = 16, 128
NCORES = 8
HL = H // NCORES          # heads per core = 2
TT = B * T                # 4096
NKC = C // 128            # 16 contraction chunks
TW = 512                  # t-window (psum bank width in f32)
TW2 = 1024                # block tq width (2 banks)
NTWB = T // TW            # 4 x-windows per batch
NW = B * NTWB             # 8 windows total
NTC = T // 128            # 16 tk chunks per batch
TS = 128                  # per-core token slice per (b,hf) quarter
SCALE = float(1.0 / np.sqrt(D))

_CACHE = {}


def _build():
    from concourse import bacc, mybir, tile

    f32 = mybir.dt.float32
    f16 = mybir.dt.float16
    EXP = mybir.ActivationFunctionType.Exp

    nc = bacc.Bacc("TRN2", target_bir_lowering=False, debug=False,
                   num_devices=NCORES)

    # host pre-chunked layouts: per-partition-contiguous, no DMA rearranges
    xPC = nc.dram_tensor("xPC", [NW, 128, NKC * TW], f16, kind="ExternalInput")
    wqkPC = nc.dram_tensor("wqkPC", [128, NKC * 512], f16, kind="ExternalInput")
    wvPC = nc.dram_tensor("wvPC", [128, NKC * 256], f16, kind="ExternalInput")
    wpPC = nc.dram_tensor("wpPC", [128, NKC * 2048], f16,
                          kind="ExternalInput")
    cosPC = nc.dram_tensor("cosPC", [128, T], f16, kind="ExternalInput")
    sinPC = nc.dram_tensor("sinPC", [128, T], f16, kind="ExternalInput")
    # [2048 out chans, 4 quarters x 128 owned tokens]
    outT = nc.dram_tensor("outT", [C, 2 * 2 * TS], f16, kind="ExternalOutput")

    with tile.TileContext(nc) as tc:
        with tc.tile_pool(name="dram", bufs=1, space="DRAM") as dram:
            # A2A buffers per (batch, tq-half): in = [dst core, my 2 heads x
            # 128 d, 128 t], out = [src core (=head pair), 256, 128]
            y2 = [[dram.tile([NCORES, HL * 128, TS], f16, tag=f"y{b}{hf}",
                             name=f"y{b}{hf}") for hf in range(2)]
                  for b in range(B)]
            ya2 = [[dram.tile([NCORES, HL * 128, TS], f16, tag=f"ya{b}{hf}",
                              name=f"ya{b}{hf}")
                    for hf in range(2)] for b in range(B)]

            with (
                # PSUM tags (8 banks): sc [128,1024]x2 (scores + denom slots),
                # py [128,1024]x1 (attn@v accum / window-0 mi2+mi3),
                # w [128,512]x2 (qk-proj, v-proj, out-proj streams)
                tc.tile_pool(name="psum", bufs=2, space="PSUM") as psum,
                tc.tile_pool(name="pB", bufs=1) as pB,
            ):
                v_sb = pB.tile([128, B * NTC, HL * 128], f16, tag="v")
                # q_h0, q_h1, k_h0, k_h1 per batch, SBUF-resident
                qk_sb = [[pB.tile([128, T], f16, tag=f"qk{b}{mi}",
                                  name=f"qk{b}{mi}") for mi in range(4)]
                         for b in range(B)]
                ones16 = pB.tile([128, 1], f16, tag="ones16")
                nc.vector.memset(ones16[:], 1.0)
                # Wproj out-chans 0-1023, loaded mid-front (oc-major chunks);
                # the other half lives in the back-half pool
                wp_lo = pB.tile([128, 8 * NKC, 128], f16, tag="wplo")

                pA_cm = tc.tile_pool(name="pA", bufs=1)
                pA = pA_cm.__enter__()
                pR_cm = tc.tile_pool(name="pR", bufs=1)
                pR = pR_cm.__enter__()

                # ---- phase A prologue: critical-path-ordered split DMAs ----
                wqk8 = [pA.tile([128, 2, 512], f16, tag=f"wqk{g}",
                                name=f"wqk{g}") for g in range(8)]
                x0g = [pA.tile([128, 4, TW], f16, tag=f"x0{g}",
                               name=f"x0{g}") for g in range(4)]
                cos_sb = pA.tile([128, T], f16, tag="cos")
                sin_sb = pA.tile([128, T], f16, tag="sin")
                wv_sb = pA.tile([128, NKC, 256], f16, tag="wv")

                def dma_wqk(g):
                    nc.sync.dma_start(wqk8[g][:],
                                      wqkPC[:, g * 2 * 512:(g + 1) * 2 * 512]
                                      .rearrange("p (kc o) -> p kc o", kc=2))

                def dma_x0(g):
                    nc.sync.dma_start(x0g[g][:],
                                      xPC[0, :, g * 4 * TW:(g + 1) * 4 * TW]
                                      .rearrange("p (kc t) -> p kc t", kc=4))

                # interleaved by first-use order (kc-outer window 0)
                dma_wqk(0); dma_x0(0); dma_wqk(1); dma_x0(1)
                dma_wqk(2); dma_wqk(3); dma_x0(2)
                dma_wqk(4); dma_wqk(5); dma_x0(3)
                dma_wqk(6); dma_wqk(7)
                nc.sync.dma_start(cos_sb[:], cosPC[:])
                nc.sync.dma_start(sin_sb[:], sinPC[:])
                nc.sync.dma_start(wv_sb[:],
                                  wvPC[:].rearrange("p (kc o) -> p kc o",
                                                    kc=NKC))

                xtiles = {}

                def xacc(tw):
                    if tw == 0:
                        return lambda kc: x0g[kc // 4][:, kc % 4, :]
                    t = xtiles[tw]
                    return lambda kc: t[:, kc, :]

                def wqkat(kc, mi):
                    return wqk8[kc // 2][:, kc % 2, mi * 128:(mi + 1) * 128]

                def prefetch_x(tw):
                    t = pA.tile([128, NKC, TW], f16, tag="x", bufs=2,
                                name="x_sb")
                    nc.sync.dma_start(
                        t[:], xPC[tw].rearrange("p (kc t) -> p kc t", kc=NKC))
                    xtiles[tw] = t

                def rope(b, mi, src_ap, cs):
                    """q' = q*cos + swap_halves(q)*sin_signed, into qk_sb."""
                    qraw = pR.tile([128, TW], f16, tag="qraw", bufs=2,
                                   name="qraw")
                    nc.scalar.copy(qraw[:], src_ap)
                    qrot = pR.tile([128, TW], f16, tag="qrot", bufs=2,
                                   name="qrot")
                    nc.sync.dma_start(qrot[0:64, :], qraw[64:128, :])
                    nc.sync.dma_start(qrot[64:128, :], qraw[0:64, :])
                    dst = qk_sb[b][mi][:, cs]
                    nc.vector.tensor_mul(dst, qraw[:], cos_sb[:, cs])
                    nc.vector.tensor_mul(qrot[:], qrot[:], sin_sb[:, cs])
                    nc.vector.tensor_add(dst, dst, qrot[:])

                def win0():
                    """window 0, kc-outer: streams against the split DMAs."""
                    xat = xacc(0)
                    cs = slice(0, TW)
                    pq = [psum.tile([128, TW], f32, tag="w", name="pq01")
                          for _ in range(2)]
                    pcd = psum.tile([128, TW2], f32, tag="py", bufs=1,
                                    name="pq23")
                    mi_ap = [pq[0][:], pq[1][:], pcd[:, 0:TW], pcd[:, TW:]]
                    for kc in range(NKC):
                        for mi in range(4):
                            nc.tensor.matmul(
                                mi_ap[mi], wqkat(kc, mi), xat(kc),
                                start=(kc == 0), stop=(kc == NKC - 1))
                        if kc == 8:
                            prefetch_x(1)
                    for mi in range(4):
                        rope(0, mi, mi_ap[mi], cs)
                    for tci in range(4):
                        pv = psum.tile([128, TW], f32, tag="w", name="pv")
                        for kc in range(NKC):
                            nc.tensor.matmul(
                                pv[:, 0:256],
                                xat(kc)[:, tci * 128:(tci + 1) * 128],
                                wv_sb[:, kc, :],
                                start=(kc == 0), stop=(kc == NKC - 1))
                        nc.vector.tensor_copy(v_sb[:, tci, :], pv[:, 0:256])

                def win_gen(tw):
                    """QKV projection + rope for one 512-wide t window.
                    8 yields: 4 q/k mi-streams, 4 v tci-streams."""
                    b, twb = divmod(tw, NTWB)
                    xat = xacc(tw)
                    cs = slice(twb * TW, (twb + 1) * TW)
                    for mi in range(4):
                        pqk = psum.tile([128, TW], f32, tag="w", name="pqk")
                        for kc in range(NKC):
                            nc.tensor.matmul(
                                pqk[:], wqkat(kc, mi), xat(kc),
                                start=(kc == 0), stop=(kc == NKC - 1))
                        rope(b, mi, pqk[:], cs)
                        if mi == 0 and tw + 1 < NW:
                            prefetch_x(tw + 1)
                        yield
                    for tci in range(4):
                        pv = psum.tile([128, TW], f32, tag="w", name="pv")
                        for kc in range(NKC):
                            nc.tensor.matmul(
                                pv[:, 0:256],
                                xat(kc)[:, tci * 128:(tci + 1) * 128],
                                wv_sb[:, kc, :],
                                start=(kc == 0), stop=(kc == NKC - 1))
                        nc.vector.tensor_copy(v_sb[:, tw * 4 + tci, :],
                                              pv[:, 0:256])
                        yield

                def attn_gen(b, hf, h):
                    """scoresT+softmax+attn@v for one (batch, tq-half, head).
                    17 yields: 16 pipelined tkc units + tail (A2A on h==1)."""
                    qh, kh = qk_sb[b][h], qk_sb[b][2 + h]
                    ssum = pB.tile([128, TW2], f16, tag="ssum", bufs=2,
                                   name="ssum")
                    py = psum.tile([128, TW2], f32, tag="py", bufs=1,
                                   name="py")
                    es = []

                    def attnv(i):
                        for j in range(2):
                            nc.tensor.matmul(
                                py[:, j * TW:(j + 1) * TW],
                                v_sb[:, b * NTC + i, h * 128:(h + 1) * 128],
                                es[i][:, j * TW:(j + 1) * TW],
                                start=(i == 0), stop=(i == NTC - 1))

                    for tkc in range(NTC):
                        sc = psum.tile([128, TW2], f32, tag="sc", name="sc")
                        for j in range(2):
                            tq0 = hf * TW2 + j * TW
                            nc.tensor.matmul(
                                sc[:, j * TW:(j + 1) * TW],
                                kh[:, tkc * 128:(tkc + 1) * 128],
                                qh[:, tq0:tq0 + TW],
                                start=True, stop=True)
                        e = pB.tile([128, TW2], f16, tag="e", bufs=4,
                                    name="e")
                        es.append(e)
                        nc.scalar.activation(e[:], sc[:], EXP, scale=SCALE)
                        if tkc == 0:
                            nc.vector.tensor_copy(ssum[:], e[:])
                        else:
                            nc.vector.tensor_add(ssum[:], ssum[:], e[:])
                        if tkc >= 2:
                            attnv(tkc - 2)
                        yield
                    attnv(NTC - 2)
                    attnv(NTC - 1)
                    # denominator: partition-reduce ssum via ones-matmuls into
                    # a rotating sc slot, then approx-reciprocal + broadcast
                    dn = psum.tile([128, TW2], f32, tag="sc", name="dn")
                    for j in range(2):
                        nc.tensor.matmul(dn[0:1, j * TW:(j + 1) * TW],
                                         ones16[:],
                                         ssum[:, j * TW:(j + 1) * TW],
                                         start=True, stop=True)
                    rc = pB.tile([1, TW2], f32, tag="rc", bufs=2, name="rc")
                    nc.vector.reciprocal_approx_fast(rc[:], dn[0:1, :])
                    rbs = pB.tile([128, TW2], f32, tag="rbs", bufs=2,
                                  name="rbs")
                    nc.gpsimd.partition_broadcast(rbs[:], rc[:])
                    ybf = pB.tile([128, TW2], f16, tag="ybf", bufs=2,
                                  name="ybf")
                    for j in range(2):
                        nc.vector.tensor_mul(ybf[:, j * TW:(j + 1) * TW],
                                             py[:, j * TW:(j + 1) * TW],
                                             rbs[:, j * TW:(j + 1) * TW])
                    # scatter into the A2A src layout [dst core, 256, 128]
                    nc.sync.dma_start(
                        y2[b][hf][:]
                        .rearrange("k (hh d) t -> hh d k t", hh=HL)[h],
                        ybf[:].rearrange("d (k t) -> d k t", k=NCORES))
                    if h == HL - 1:
                        nc.gpsimd.collective_compute(
                            "AllToAll",
                            mybir.AluOpType.bypass,
                            replica_groups=[list(range(NCORES))],
                            ins=[y2[b][hf][:]],
                            outs=[ya2[b][hf][:]],
                        )
                    yield

                def drive(gen, n=None):
                    if n is None:
                        for _ in gen:
                            pass
                    else:
                        for _ in range(n):
                            next(gen)

                # ---- front half ------------------------------------------
                win0()
                for twb in range(1, NTWB):
                    drive(win_gen(twb))
                for i in range(NTWB):
                    w = win_gen(NTWB + i)
                    a = attn_gen(0, i // 2, i % 2)
                    for _ in range(8):
                        drive(w, 1)
                        drive(a, 2)
                    drive(a)
                    # stage the Wproj lo-half load behind the startup traffic
                    if i in (0, 1):
                        nq = 4 * NKC          # 4 oc chunks per DMA
                        nc.sync.dma_start(
                            wp_lo[:, i * nq:(i + 1) * nq, :],
                            wpPC[:, i * nq * 128:(i + 1) * nq * 128]
                            .rearrange("p (c o) -> p c o", c=nq))

                pR_cm.__exit__(None, None, None)
                pA_cm.__exit__(None, None, None)

                # ---- back half -------------------------------------------
                with tc.tile_pool(name="pC", bufs=1) as pC:
                    wp_hi = pC.tile([128, 8 * NKC, 128], f16, tag="wphi")
                    HIO = 8 * NKC * 128
                    for g in range(2):
                        nq = 4 * NKC
                        nc.sync.dma_start(
                            wp_hi[:, g * nq:(g + 1) * nq, :],
                            wpPC[:, HIO + g * nq * 128:HIO + (g + 1) * nq * 128]
                            .rearrange("p (c o) -> p c o", c=nq))

                    def wpat(oc, kc):
                        if oc < 8:
                            return wp_lo[:, oc * NKC + kc, :]
                        return wp_hi[:, (oc - 8) * NKC + kc, :]

                    yq = {}

                    def load_ya(b, hf):
                        t = pC.tile([128, NKC, TS], f16, tag="yq", bufs=2,
                                    name="yq")
                        nc.sync.dma_start(
                            t[:],
                            ya2[b][hf][:]
                            .rearrange("s (hh p) t -> p (s hh) t", p=128))
                        yq[(b, hf)] = t

                    def proj_gen(b, hf):
                        """all 2048 out chans for my 128 owned tokens of one
                        (b,hf) quarter; 16 yields (one per oc chunk)."""
                        yt = yq[(b, hf)]
                        q = 2 * b + hf
                        for oc in range(NKC):
                            po = psum.tile([128, TW], f32, tag="w",
                                           name="po")
                            for kc in range(NKC):
                                nc.tensor.matmul(
                                    po[:, 0:TS],
                                    wpat(oc, kc),
                                    yt[:, kc, :],
                                    start=(kc == 0), stop=(kc == NKC - 1))
                            od = pC.tile([128, TS], f16, tag="od", bufs=2,
                                         name="od")
                            nc.vector.tensor_copy(od[:], po[:, 0:TS])
                            nc.sync.dma_start(
                                outT[oc * 128:(oc + 1) * 128,
                                     q * TS:(q + 1) * TS],
                                od[:])
                            yield

                    load_ya(0, 0)
                    load_ya(0, 1)
                    weave = [(0, 0), (0, 0), (0, 1), (0, 1)]
                    pgen = {}
                    for i in range(4):
                        a = attn_gen(1, i // 2, i % 2)
                        key = weave[i]
                        if key not in pgen:
                            pgen[key] = proj_gen(*key)
                        p = pgen[key]
                        for u in range(NTC):
                            drive(a, 1)
                            if u % 2 == 1:
                                drive(p, 1)
                        drive(a)
                        if i == 1:
                            load_ya(1, 0)
                        elif i == 3:
                            load_ya(1, 1)
                    drive(proj_gen(1, 0))
                    drive(proj_gen(1, 1))
    nc.compile()
    return nc


def _prepare_in_maps(x, cos, sin, Wqkv, Wproj):
    f16 = np.float16

    def chunk(a):
        # [NKC*128, N] -> [128, NKC*N] per-partition-contiguous kc-major
        n = a.shape[1]
        return np.ascontiguousarray(
            a.reshape(NKC, 128, n).transpose(1, 0, 2).reshape(128, NKC * n))

    xT = x.reshape(TT, C).T.astype(f16)                      # [C, TT]
    xPC = np.empty((NW, 128, NKC * TW), dtype=f16)
    for tw in range(NW):
        xPC[tw] = chunk(xT[:, tw * TW:(tw + 1) * TW])
    cosPC = np.ascontiguousarray(cos.T).astype(f16)          # [128, T]
    sinS = sin.T.astype(np.float32).copy()
    sinS[:D // 2] *= -1.0
    sinPC = np.ascontiguousarray(sinS).astype(f16)
    Wq, Wk, Wv = Wqkv[0:C], Wqkv[C:2 * C], Wqkv[2 * C:3 * C]
    # Wproj oc-major: [p, (oc kc o)] with block(oc,kc) = WT[kc*128:, oc*128:]
    wpPC = np.ascontiguousarray(
        Wproj.T.astype(f16).reshape(NKC, 128, NKC, 128)
        .transpose(1, 2, 0, 3).reshape(128, NKC * NKC * 128))

    in_maps = []
    for c in range(NCORES):
        hs = [HL * c + j for j in range(HL)]
        wqk_rows = np.concatenate(
            [Wq[h * D:(h + 1) * D] for h in hs]
            + [Wk[h * D:(h + 1) * D] for h in hs], axis=0)
        wv_rows = np.concatenate([Wv[h * D:(h + 1) * D] for h in hs], axis=0)
        in_maps.append({
            "xPC": xPC,
            "wqkPC": chunk(wqk_rows.T.astype(f16)),
            "wvPC": chunk(wv_rows.T.astype(f16)),
            "wpPC": wpPC,
            "cosPC": cosPC,
            "sinPC": sinPC,
        })
    return in_maps


def run_sharded(x, cos, sin, Wqkv, Wproj, trace=False):
    """Compile (cached), run on 8 cores, return (out, BassKernelResults)."""
    from concourse.bass_utils import run_bass_kernel_spmd

    if "nc" not in _CACHE:
        _CACHE["nc"] = _build()
    nc = _CACHE["nc"]
    in_maps = _prepare_in_maps(np.asarray(x), np.asarray(cos),
                               np.asarray(sin), np.asarray(Wqkv),
                               np.asarray(Wproj))
    res = run_bass_kernel_spmd(nc, in_maps, core_ids=list(range(NCORES)),
                               trace=trace)
    out = np.empty((B, T, C), dtype=np.float32)
    for c in range(NCORES):
        # outT [2048, 4*128]: col q*128+tl is token (b, hf*1024+c*128+tl)
        oT = res.results[c]["outT"].astype(np.float32)
        oT = oT.reshape(C, 2, 2, TS)          # [oc, b, hf, tl]
        for b in range(B):
            for hf in range(2):
                t0 = hf * TW2 + c * TS
                out[b, t0:t0 + TS, :] = oT[:, b, hf, :].T
    return out, res


def kernel(x, cos, sin, Wqkv, Wproj):
    out, _ = run_sharded(x, cos, sin, Wqkv, Wproj, trace=False)
    return out


# revision 16
# speedup vs baseline: 1.2463x; 1.0024x over previous
"""Distributed Trainium2 kernel for a full attention block (QKV proj + RoPE +
bidirectional SDPA + output proj), SPMD across 8 NeuronCores.

Sharding: tensor-parallel over heads (16 heads -> 2 per core) for QKV+attention.
The merge for the output projection uses 4 AllToAlls (one per batch x tq-half
quarter): each core keeps a 128-token slice of every quarter and computes ALL
2048 output channels for its 512 owned tokens. A2A moves each y element once
(~0.45MB wire/core per quarter) vs AllGather's 8x (every element to every
core), which un-bottlenecks the serial CC stream that dominated the v2 tail.

v3 structure (v2 measured 509us, v1 539-584us; PE floor ~402us at the
observed ~0.5ns/col stream rate):
  - front half: 4 batch-0 QKV/RoPE windows (window 0 is kc-outer so the first
    matmul starts ~3us in, streaming against the split weight/x DMAs), then 4
    slots of batch-1 window || batch-0 attention block, with the block's 16
    (scores->exp->ssum->attn@v[skew-2]) units woven between the window's 8
    matmul streams so the in-order PE never waits on the Scalar-engine exp.
  - A2A(0,hf) fires mid-front; A2A(1,hf) fires as back-half blocks finish.
  - back half: 4 batch-1 blocks with batch-0 proj quarters woven between
    units; batch-1 proj runs last, its A2A wait hidden under the (1,0) proj.
  - q,k SBUF-resident; softmax denominator = f16 SBUF DVE accumulation +
    ones-matmul partition reduction + reciprocal_approx_fast + gpsimd
    broadcast.

dtypes: f16 everywhere on the matmul/exp/collective path, f32 psum, f16
output (host converts); full Wproj (8MB f16) is SBUF-resident per core.
"""
import sys
for _p in ("/opt/trn_rl_repo",):
    if _p not in sys.path:
        sys.path.append(_p)

import numpy as np

B, T, C = 2, 2048, 2048
H, D = 16, 128
NCORES = 8
HL = H // NCORES          # heads per core = 2
TT = B * T                # 4096
NKC = C // 128            # 16 contraction chunks
TW = 512                  # t-window (psum bank width in f32)
TW2 = 1024                # block tq width (2 banks)
NTWB = T // TW            # 4 x-windows per batch
NW = B * NTWB             # 8 windows total
NTC = T // 128            # 16 tk chunks per batch
TS = 128                  # per-core token slice per (b,hf) quarter
SCALE = float(1.0 / np.sqrt(D))

_CACHE = {}


def _build():
    from concourse import bacc, mybir, tile

    f32 = mybir.dt.float32
    f16 = mybir.dt.float16
    EXP = mybir.ActivationFunctionType.Exp

    nc = bacc.Bacc("TRN2", target_bir_lowering=False, debug=False,
                   num_devices=NCORES)

    # host pre-chunked layouts: per-partition-contiguous, no DMA rearranges
    xPC = nc.dram_tensor("xPC", [NW, 128, NKC * TW], f16, kind="ExternalInput")
    wqkPC = nc.dram_tensor("wqkPC", [128, NKC * 512], f16, kind="ExternalInput")
    wvPC = nc.dram_tensor("wvPC", [128, NKC * 256], f16, kind="ExternalInput")
    wpPC = nc.dram_tensor("wpPC", [128, NKC * 2048], f16,
                          kind="ExternalInput")
    cosPC = nc.dram_tensor("cosPC", [128, T], f16, kind="ExternalInput")
    sinPC = nc.dram_tensor("sinPC", [128, T], f16, kind="ExternalInput")
    # [2048 out chans, 4 quarters x 128 owned tokens]
    outT = nc.dram_tensor("outT", [C, 2 * 2 * TS], f16, kind="ExternalOutput")

    with tile.TileContext(nc) as tc:
        with tc.tile_pool(name="dram", bufs=1, space="DRAM") as dram:
            # A2A buffers per (batch, tq-half): in = [dst core, my 2 heads x
            # 128 d, 128 t], out = [src core (=head pair), 256, 128]
            y2 = [[dram.tile([NCORES, HL * 128, TS], f16, tag=f"y{b}{hf}",
                             name=f"y{b}{hf}") for hf in range(2)]
                  for b in range(B)]
            ya2 = [[dram.tile([NCORES, HL * 128, TS], f16, tag=f"ya{b}{hf}",
                              name=f"ya{b}{hf}")
                    for hf in range(2)] for b in range(B)]

            with (
                # PSUM tags (8 banks): sc [128,1024]x2 (scores + denom slots),
                # py [128,1024]x1 (attn@v accum / window-0 mi2+mi3),
                # w [128,512]x2 (qk-proj, v-proj, out-proj streams)
                tc.tile_pool(name="psum", bufs=2, space="PSUM") as psum,
                tc.tile_pool(name="pB", bufs=1) as pB,
            ):
                v_sb = pB.tile([128, B * NTC, HL * 128], f16, tag="v")
                # q_h0, q_h1, k_h0, k_h1 per batch, SBUF-resident
                qk_sb = [[pB.tile([128, T], f16, tag=f"qk{b}{mi}",
                                  name=f"qk{b}{mi}") for mi in range(4)]
                         for b in range(B)]
                ones16 = pB.tile([128, 1], f16, tag="ones16")
                nc.vector.memset(ones16[:], 1.0)
                # Wproj out-chans 0-1023, loaded mid-front (oc-major chunks);
                # the other half lives in the back-half pool
                wp_lo = pB.tile([128, 8 * NKC, 128], f16, tag="wplo")

                pA_cm = tc.tile_pool(name="pA", bufs=1)
                pA = pA_cm.__enter__()
                pR_cm = tc.tile_pool(name="pR", bufs=1)
                pR = pR_cm.__enter__()

                # ---- phase A prologue: critical-path-ordered split DMAs ----
                wqk8 = [pA.tile([128, 2, 512], f16, tag=f"wqk{g}",
                                name=f"wqk{g}") for g in range(8)]
                x0g = [pA.tile([128, 4, TW], f16, tag=f"x0{g}",
                               name=f"x0{g}") for g in range(4)]
                cos_sb = pA.tile([128, T], f16, tag="cos")
                sin_sb = pA.tile([128, T], f16, tag="sin")
                wv_sb = pA.tile([128, NKC, 256], f16, tag="wv")

                def dma_wqk(g):
                    nc.sync.dma_start(wqk8[g][:],
                                      wqkPC[:, g * 2 * 512:(g + 1) * 2 * 512]
                                      .rearrange("p (kc o) -> p kc o", kc=2))

                def dma_x0(g):
                    nc.sync.dma_start(x0g[g][:],
                                      xPC[0, :, g * 4 * TW:(g + 1) * 4 * TW]
                                      .rearrange("p (kc t) -> p kc t", kc=4))

                # interleaved by first-use order (kc-outer window 0)
                dma_wqk(0); dma_x0(0); dma_wqk(1); dma_x0(1)
                dma_wqk(2); dma_wqk(3); dma_x0(2)
                dma_wqk(4); dma_wqk(5); dma_x0(3)
                dma_wqk(6); dma_wqk(7)
                nc.sync.dma_start(cos_sb[:], cosPC[:])
                nc.sync.dma_start(sin_sb[:], sinPC[:])
                nc.sync.dma_start(wv_sb[:],
                                  wvPC[:].rearrange("p (kc o) -> p kc o",
                                                    kc=NKC))

                xtiles = {}

                def xacc(tw):
                    if tw == 0:
                        return lambda kc: x0g[kc // 4][:, kc % 4, :]
                    t = xtiles[tw]
                    return lambda kc: t[:, kc, :]

                def wqkat(kc, mi):
                    return wqk8[kc // 2][:, kc % 2, mi * 128:(mi + 1) * 128]

                def prefetch_x(tw):
                    t = pA.tile([128, NKC, TW], f16, tag="x", bufs=2,
                                name="x_sb")
                    nc.sync.dma_start(
                        t[:], xPC[tw].rearrange("p (kc t) -> p kc t", kc=NKC))
                    xtiles[tw] = t

                def rope(b, mi, src_ap, cs):
                    """q' = q*cos + swap_halves(q)*sin_signed, into qk_sb."""
                    qraw = pR.tile([128, TW], f16, tag="qraw", bufs=2,
                                   name="qraw")
                    nc.scalar.copy(qraw[:], src_ap)
                    qrot = pR.tile([128, TW], f16, tag="qrot", bufs=2,
                                   name="qrot")
                    nc.sync.dma_start(qrot[0:64, :], qraw[64:128, :])
                    nc.sync.dma_start(qrot[64:128, :], qraw[0:64, :])
                    dst = qk_sb[b][mi][:, cs]
                    nc.vector.tensor_mul(dst, qraw[:], cos_sb[:, cs])
                    nc.vector.tensor_mul(qrot[:], qrot[:], sin_sb[:, cs])
                    nc.vector.tensor_add(dst, dst, qrot[:])

                def win0():
                    """window 0, kc-outer: streams against the split DMAs."""
                    xat = xacc(0)
                    cs = slice(0, TW)
                    pq = [psum.tile([128, TW], f32, tag="w", name="pq01")
                          for _ in range(2)]
                    pcd = psum.tile([128, TW2], f32, tag="py", bufs=1,
                                    name="pq23")
                    mi_ap = [pq[0][:], pq[1][:], pcd[:, 0:TW], pcd[:, TW:]]
                    for kc in range(NKC):
                        for mi in range(4):
                            nc.tensor.matmul(
                                mi_ap[mi], wqkat(kc, mi), xat(kc),
                                start=(kc == 0), stop=(kc == NKC - 1))
                        if kc == 8:
                            prefetch_x(1)
                    for mi in range(4):
                        rope(0, mi, mi_ap[mi], cs)
                    for tci in range(4):
                        pv = psum.tile([128, TW], f32, tag="w", name="pv")
                        for kc in range(NKC):
                            nc.tensor.matmul(
                                pv[:, 0:256],
                                xat(kc)[:, tci * 128:(tci + 1) * 128],
                                wv_sb[:, kc, :],
                                start=(kc == 0), stop=(kc == NKC - 1))
                        nc.vector.tensor_copy(v_sb[:, tci, :], pv[:, 0:256])

                def win_gen(tw):
                    """QKV projection + rope for one 512-wide t window.
                    8 yields: 4 q/k mi-streams, 4 v tci-streams."""
                    b, twb = divmod(tw, NTWB)
                    xat = xacc(tw)
                    cs = slice(twb * TW, (twb + 1) * TW)
                    for mi in range(4):
                        pqk = psum.tile([128, TW], f32, tag="w", name="pqk")
                        for kc in range(NKC):
                            nc.tensor.matmul(
                                pqk[:], wqkat(kc, mi), xat(kc),
                                start=(kc == 0), stop=(kc == NKC - 1))
                        rope(b, mi, pqk[:], cs)
                        if mi == 0 and tw + 1 < NW:
                            prefetch_x(tw + 1)
                        yield
                    for tci in range(4):
                        pv = psum.tile([128, TW], f32, tag="w", name="pv")
                        for kc in range(NKC):
                            nc.tensor.matmul(
                                pv[:, 0:256],
                                xat(kc)[:, tci * 128:(tci + 1) * 128],
                                wv_sb[:, kc, :],
                                start=(kc == 0), stop=(kc == NKC - 1))
                        nc.vector.tensor_copy(v_sb[:, tw * 4 + tci, :],
                                              pv[:, 0:256])
                        yield

                def attn_gen(b, hf, h):
                    """scoresT+softmax+attn@v for one (batch, tq-half, head).
                    17 yields: 16 pipelined tkc units + tail (A2A on h==1)."""
                    qh, kh = qk_sb[b][h], qk_sb[b][2 + h]
                    ssum = pB.tile([128, TW2], f16, tag="ssum", bufs=2,
                                   name="ssum")
                    py = psum.tile([128, TW2], f32, tag="py", bufs=1,
                                   name="py")
                    es = []

                    def attnv(i):
                        for j in range(2):
                            nc.tensor.matmul(
                                py[:, j * TW:(j + 1) * TW],
                                v_sb[:, b * NTC + i, h * 128:(h + 1) * 128],
                                es[i][:, j * TW:(j + 1) * TW],
                                start=(i == 0), stop=(i == NTC - 1))

                    for tkc in range(NTC):
                        sc = psum.tile([128, TW2], f32, tag="sc", name="sc")
                        for j in range(2):
                            tq0 = hf * TW2 + j * TW
                            nc.tensor.matmul(
                                sc[:, j * TW:(j + 1) * TW],
                                kh[:, tkc * 128:(tkc + 1) * 128],
                                qh[:, tq0:tq0 + TW],
                                start=True, stop=True)
                        e = pB.tile([128, TW2], f16, tag="e", bufs=4,
                                    name="e")
                        es.append(e)
                        nc.scalar.activation(e[:], sc[:], EXP, scale=SCALE)
                        if tkc == 0:
                            nc.vector.tensor_copy(ssum[:], e[:])
                        else:
                            nc.vector.tensor_add(ssum[:], ssum[:], e[:])
                        if tkc >= 2:
                            attnv(tkc - 2)
                        yield
                    attnv(NTC - 2)
                    attnv(NTC - 1)
                    # denominator: partition-reduce ssum via ones-matmuls into
                    # a rotating sc slot, then approx-reciprocal + broadcast
                    dn = psum.tile([128, TW2], f32, tag="sc", name="dn")
                    for j in range(2):
                        nc.tensor.matmul(dn[0:1, j * TW:(j + 1) * TW],
                                         ones16[:],
                                         ssum[:, j * TW:(j + 1) * TW],
                                         start=True, stop=True)
                    rc = pB.tile([1, TW2], f32, tag="rc", bufs=2, name="rc")
                    nc.vector.reciprocal_approx_fast(rc[:], dn[0:1, :])
                    rbs = pB.tile([128, TW2], f32, tag="rbs", bufs=2,
                                  name="rbs")
                    nc.gpsimd.partition_broadcast(rbs[:], rc[:])
                    ybf = pB.tile([128, TW2], f16, tag="ybf", bufs=2,
                                  name="ybf")
                    for j in range(2):
                        nc.vector.tensor_mul(ybf[:, j * TW:(j + 1) * TW],
                                             py[:, j * TW:(j + 1) * TW],
                                             rbs[:, j * TW:(j + 1) * TW])
                    # scatter into the A2A src layout [dst core, 256, 128];
                    # issued from ACT so the trigger isn't queued behind Sync
                    nc.scalar.dma_start(
                        y2[b][hf][:]
                        .rearrange("k (hh d) t -> hh d k t", hh=HL)[h],
                        ybf[:].rearrange("d (k t) -> d k t", k=NCORES))
                    if h == HL - 1:
                        nc.gpsimd.collective_compute(
                            "AllToAll",
                            mybir.AluOpType.bypass,
                            replica_groups=[list(range(NCORES))],
                            ins=[y2[b][hf][:]],
                            outs=[ya2[b][hf][:]],
                        )
                    yield

                def drive(gen, n=None):
                    if n is None:
                        for _ in gen:
                            pass
                    else:
                        for _ in range(n):
                            next(gen)

                # ---- front half ------------------------------------------
                win0()
                for twb in range(1, NTWB):
                    drive(win_gen(twb))
                for i in range(NTWB):
                    w = win_gen(NTWB + i)
                    a = attn_gen(0, i // 2, i % 2)
                    for _ in range(8):
                        drive(w, 1)
                        drive(a, 2)
                    drive(a)
                    # stage the Wproj lo-half load behind the startup traffic
                    if i in (0, 1):
                        nq = 4 * NKC          # 4 oc chunks per DMA
                        nc.sync.dma_start(
                            wp_lo[:, i * nq:(i + 1) * nq, :],
                            wpPC[:, i * nq * 128:(i + 1) * nq * 128]
                            .rearrange("p (c o) -> p c o", c=nq))

                pR_cm.__exit__(None, None, None)
                pA_cm.__exit__(None, None, None)

                # ---- back half -------------------------------------------
                with tc.tile_pool(name="pC", bufs=1) as pC:
                    wp_hi = pC.tile([128, 8 * NKC, 128], f16, tag="wphi")
                    HIO = 8 * NKC * 128
                    for g in range(2):
                        nq = 4 * NKC
                        nc.sync.dma_start(
                            wp_hi[:, g * nq:(g + 1) * nq, :],
                            wpPC[:, HIO + g * nq * 128:HIO + (g + 1) * nq * 128]
                            .rearrange("p (c o) -> p c o", c=nq))

                    def wpat(oc, kc):
                        if oc < 8:
                            return wp_lo[:, oc * NKC + kc, :]
                        return wp_hi[:, (oc - 8) * NKC + kc, :]

                    yq = {}

                    def load_ya(b, hf):
                        t = pC.tile([128, NKC, TS], f16, tag="yq", bufs=2,
                                    name="yq")
                        nc.sync.dma_start(
                            t[:],
                            ya2[b][hf][:]
                            .rearrange("s (hh p) t -> p (s hh) t", p=128))
                        yq[(b, hf)] = t

                    def proj_gen(b, hf):
                        """all 2048 out chans for my 128 owned tokens of one
                        (b,hf) quarter; 16 yields (one per oc chunk)."""
                        yt = yq[(b, hf)]
                        q = 2 * b + hf
                        for oc in range(NKC):
                            po = psum.tile([128, TW], f32, tag="w",
                                           name="po")
                            for kc in range(NKC):
                                nc.tensor.matmul(
                                    po[:, 0:TS],
                                    wpat(oc, kc),
                                    yt[:, kc, :],
                                    start=(kc == 0), stop=(kc == NKC - 1))
                            od = pC.tile([128, TS], f16, tag="od", bufs=2,
                                         name="od")
                            if oc % 2 == 0:
                                nc.vector.tensor_copy(od[:], po[:, 0:TS])
                            else:
                                nc.scalar.copy(od[:], po[:, 0:TS])
                            nc.sync.dma_start(
                                outT[oc * 128:(oc + 1) * 128,
                                     q * TS:(q + 1) * TS],
                                od[:])
                            yield

                    # weave only 4 proj chunks per block slot; the ~35us of
                    # remaining collective-independent proj work then covers
                    # the last A2A's latency after the final block
                    load_ya(0, 0)
                    load_ya(0, 1)
                    weave = [(0, 0), (0, 0), (0, 1), (0, 1)]
                    pgen = {}
                    for i in range(4):
                        a = attn_gen(1, i // 2, i % 2)
                        key = weave[i]
                        if key not in pgen:
                            pgen[key] = proj_gen(*key)
                        p = pgen[key]
                        for u in range(NTC):
                            drive(a, 1)
                            if u % 4 == 3:
                                drive(p, 1)
                        drive(a)
                        if i == 1:
                            load_ya(1, 0)
                        elif i == 3:
                            load_ya(1, 1)
                    drive(pgen[(0, 0)])
                    drive(pgen[(0, 1)])
                    drive(proj_gen(1, 0))
                    drive(proj_gen(1, 1))
    nc.compile()
    return nc


def _prepare_in_maps(x, cos, sin, Wqkv, Wproj):
    f16 = np.float16

    def chunk(a):
        # [NKC*128, N] -> [128, NKC*N] per-partition-contiguous kc-major
        n = a.shape[1]
        return np.ascontiguousarray(
            a.reshape(NKC, 128, n).transpose(1, 0, 2).reshape(128, NKC * n))

    xT = x.reshape(TT, C).T.astype(f16)                      # [C, TT]
    xPC = np.empty((NW, 128, NKC * TW), dtype=f16)
    for tw in range(NW):
        xPC[tw] = chunk(xT[:, tw * TW:(tw + 1) * TW])
    cosPC = np.ascontiguousarray(cos.T).astype(f16)          # [128, T]
    sinS = sin.T.astype(np.float32).copy()
    sinS[:D // 2] *= -1.0
    sinPC = np.ascontiguousarray(sinS).astype(f16)
    Wq, Wk, Wv = Wqkv[0:C], Wqkv[C:2 * C], Wqkv[2 * C:3 * C]
    # Wproj oc-major: [p, (oc kc o)] with block(oc,kc) = WT[kc*128:, oc*128:]
    wpPC = np.ascontiguousarray(
        Wproj.T.astype(f16).reshape(NKC, 128, NKC, 128)
        .transpose(1, 2, 0, 3).reshape(128, NKC * NKC * 128))

    in_maps = []
    for c in range(NCORES):
        hs = [HL * c + j for j in range(HL)]
        wqk_rows = np.concatenate(
            [Wq[h * D:(h + 1) * D] for h in hs]
            + [Wk[h * D:(h + 1) * D] for h in hs], axis=0)
        wv_rows = np.concatenate([Wv[h * D:(h + 1) * D] for h in hs], axis=0)
        in_maps.append({
            "xPC": xPC,
            "wqkPC": chunk(wqk_rows.T.astype(f16)),
            "wvPC": chunk(wv_rows.T.astype(f16)),
            "wpPC": wpPC,
            "cosPC": cosPC,
            "sinPC": sinPC,
        })
    return in_maps


def run_sharded(x, cos, sin, Wqkv, Wproj, trace=False):
    """Compile (cached), run on 8 cores, return (out, BassKernelResults)."""
    from concourse.bass_utils import run_bass_kernel_spmd

    if "nc" not in _CACHE:
        _CACHE["nc"] = _build()
    nc = _CACHE["nc"]
    in_maps = _prepare_in_maps(np.asarray(x), np.asarray(cos),
                               np.asarray(sin), np.asarray(Wqkv),
                               np.asarray(Wproj))
    res = run_bass_kernel_spmd(nc, in_maps, core_ids=list(range(NCORES)),
                               trace=trace)
    out = np.empty((B, T, C), dtype=np.float32)
    for c in range(NCORES):
        # outT [2048, 4*128]: col q*128+tl is token (b, hf*1024+c*128+tl)
        oT = res.results[c]["outT"].astype(np.float32)
        oT = oT.reshape(C, 2, 2, TS)          # [oc, b, hf, tl]
        for b in range(B):
            for hf in range(2):
                t0 = hf * TW2 + c * TS
                out[b, t0:t0 + TS, :] = oT[:, b, hf, :].T
    return out, res


def kernel(x, cos, sin, Wqkv, Wproj):
    out, _ = run_sharded(x, cos, sin, Wqkv, Wproj, trace=False)
    return out


# revision 20
# speedup vs baseline: 1.2600x; 1.0110x over previous
"""Distributed Trainium2 kernel for a full attention block (QKV proj + RoPE +
bidirectional SDPA + output proj), SPMD across 8 NeuronCores.

Sharding: tensor-parallel over heads (16 heads -> 2 per core) for QKV+attention.
The merge for the output projection uses 4 AllToAlls (one per batch x tq-half
quarter): each core keeps a 128-token slice of every quarter and computes ALL
2048 output channels for its 512 owned tokens. A2A moves each y element once
(~0.45MB wire/core per quarter) vs AllGather's 8x (every element to every
core), which un-bottlenecks the serial CC stream that dominated the v2 tail.

v3 structure (v2 measured 509us, v1 539-584us; PE floor ~402us at the
observed ~0.5ns/col stream rate):
  - front half: 4 batch-0 QKV/RoPE windows (window 0 is kc-outer so the first
    matmul starts ~3us in, streaming against the split weight/x DMAs), then 4
    slots of batch-1 window || batch-0 attention block, with the block's 16
    (scores->exp->ssum->attn@v[skew-2]) units woven between the window's 8
    matmul streams so the in-order PE never waits on the Scalar-engine exp.
  - A2A(0,hf) fires mid-front; A2A(1,hf) fires as back-half blocks finish.
  - back half: 4 batch-1 blocks with batch-0 proj quarters woven between
    units; batch-1 proj runs last, its A2A wait hidden under the (1,0) proj.
  - q,k SBUF-resident; softmax denominator = f16 SBUF DVE accumulation +
    ones-matmul partition reduction + reciprocal_approx_fast + gpsimd
    broadcast.

dtypes: f16 everywhere on the matmul/exp/collective path, f32 psum, f16
output (host converts); full Wproj (8MB f16) is SBUF-resident per core.
"""
import sys
for _p in ("/opt/trn_rl_repo",):
    if _p not in sys.path:
        sys.path.append(_p)

import numpy as np

B, T, C = 2, 2048, 2048
H, D = 16, 128
NCORES = 8
HL = H // NCORES          # heads per core = 2
TT = B * T                # 4096
NKC = C // 128            # 16 contraction chunks
TW = 512                  # t-window (psum bank width in f32)
TW2 = 1024                # block tq width (2 banks)
NTWB = T // TW            # 4 x-windows per batch
NW = B * NTWB             # 8 windows total
NTC = T // 128            # 16 tk chunks per batch
TS = 128                  # per-core token slice per (b,hf) quarter
SCALE = float(1.0 / np.sqrt(D))

_CACHE = {}


def _build():
    from concourse import bacc, mybir, tile

    f32 = mybir.dt.float32
    f16 = mybir.dt.float16
    EXP = mybir.ActivationFunctionType.Exp

    nc = bacc.Bacc("TRN2", target_bir_lowering=False, debug=False,
                   num_devices=NCORES)

    # host pre-chunked layouts: per-partition-contiguous, no DMA rearranges
    xPC = nc.dram_tensor("xPC", [NW, 128, NKC * TW], f16, kind="ExternalInput")
    wqkPC = nc.dram_tensor("wqkPC", [128, NKC * 512], f16, kind="ExternalInput")
    wvPC = nc.dram_tensor("wvPC", [128, NKC * 256], f16, kind="ExternalInput")
    wpPC = nc.dram_tensor("wpPC", [128, NKC * 2048], f16,
                          kind="ExternalInput")
    cosPC = nc.dram_tensor("cosPC", [128, T], f16, kind="ExternalInput")
    sinPC = nc.dram_tensor("sinPC", [128, T], f16, kind="ExternalInput")
    # [2048 out chans, 4 quarters x 128 owned tokens]
    outT = nc.dram_tensor("outT", [C, 2 * 2 * TS], f16, kind="ExternalOutput")

    with tile.TileContext(nc) as tc:
        with tc.tile_pool(name="dram", bufs=1, space="DRAM") as dram:
            # A2A buffers per (batch, tq-half): in = [dst core, my 2 heads x
            # 128 d, 128 t], out = [src core (=head pair), 256, 128]
            y2 = [[dram.tile([NCORES, HL * 128, TS], f16, tag=f"y{b}{hf}",
                             name=f"y{b}{hf}") for hf in range(2)]
                  for b in range(B)]
            ya2 = [[dram.tile([NCORES, HL * 128, TS], f16, tag=f"ya{b}{hf}",
                              name=f"ya{b}{hf}")
                    for hf in range(2)] for b in range(B)]

            with (
                # PSUM tags (8 banks): sc [128,1024]x2 (scores + denom slots),
                # py [128,1024]x1 (attn@v accum / window-0 mi2+mi3),
                # w [128,512]x2 (qk-proj, v-proj, out-proj streams)
                tc.tile_pool(name="psum", bufs=2, space="PSUM") as psum,
                tc.tile_pool(name="pB", bufs=1) as pB,
            ):
                v_sb = pB.tile([128, B * NTC, HL * 128], f16, tag="v")
                # q_h0, q_h1, k_h0, k_h1 per batch, SBUF-resident
                qk_sb = [[pB.tile([128, T], f16, tag=f"qk{b}{mi}",
                                  name=f"qk{b}{mi}") for mi in range(4)]
                         for b in range(B)]
                ones16 = pB.tile([128, 1], f16, tag="ones16")
                nc.vector.memset(ones16[:], 1.0)
                # Wproj out-chans 0-1023, loaded mid-front (oc-major chunks);
                # the other half lives in the back-half pool
                wp_lo = pB.tile([128, 8 * NKC, 128], f16, tag="wplo")

                pA_cm = tc.tile_pool(name="pA", bufs=1)
                pA = pA_cm.__enter__()
                pR_cm = tc.tile_pool(name="pR", bufs=1)
                pR = pR_cm.__enter__()

                # ---- phase A prologue: critical-path-ordered split DMAs ----
                wqk8 = [pA.tile([128, 2, 512], f16, tag=f"wqk{g}",
                                name=f"wqk{g}") for g in range(8)]
                x0g = [pA.tile([128, 4, TW], f16, tag=f"x0{g}",
                               name=f"x0{g}") for g in range(4)]
                cos_sb = pA.tile([128, T], f16, tag="cos")
                sin_sb = pA.tile([128, T], f16, tag="sin")
                wv_sb = pA.tile([128, NKC, 256], f16, tag="wv")

                def dma_wqk(g):
                    nc.sync.dma_start(wqk8[g][:],
                                      wqkPC[:, g * 2 * 512:(g + 1) * 2 * 512]
                                      .rearrange("p (kc o) -> p kc o", kc=2))

                def dma_x0(g):
                    nc.sync.dma_start(x0g[g][:],
                                      xPC[0, :, g * 4 * TW:(g + 1) * 4 * TW]
                                      .rearrange("p (kc t) -> p kc t", kc=4))

                # interleaved by first-use order (kc-outer window 0)
                dma_wqk(0); dma_x0(0); dma_wqk(1); dma_x0(1)
                dma_wqk(2); dma_wqk(3); dma_x0(2)
                dma_wqk(4); dma_wqk(5); dma_x0(3)
                dma_wqk(6); dma_wqk(7)
                nc.sync.dma_start(wv_sb[:],
                                  wvPC[:].rearrange("p (kc o) -> p kc o",
                                                    kc=NKC))
                nc.sync.dma_start(cos_sb[:], cosPC[:])
                nc.sync.dma_start(sin_sb[:], sinPC[:])

                xtiles = {}

                def xacc(tw):
                    if tw == 0:
                        return lambda kc: x0g[kc // 4][:, kc % 4, :]
                    t = xtiles[tw]
                    return lambda kc: t[:, kc, :]

                def wqkat(kc, mi):
                    return wqk8[kc // 2][:, kc % 2, mi * 128:(mi + 1) * 128]

                def prefetch_x(tw):
                    t = pA.tile([128, NKC, TW], f16, tag="x", bufs=2,
                                name="x_sb")
                    nc.sync.dma_start(
                        t[:], xPC[tw].rearrange("p (kc t) -> p kc t", kc=NKC))
                    xtiles[tw] = t

                def rope(b, mi, src_ap, cs):
                    """q' = q*cos + swap_halves(q)*sin_signed, into qk_sb."""
                    qraw = pR.tile([128, TW], f16, tag="qraw", bufs=2,
                                   name="qraw")
                    nc.scalar.copy(qraw[:], src_ap)
                    qrot = pR.tile([128, TW], f16, tag="qrot", bufs=2,
                                   name="qrot")
                    nc.sync.dma_start(qrot[0:64, :], qraw[64:128, :])
                    nc.sync.dma_start(qrot[64:128, :], qraw[0:64, :])
                    dst = qk_sb[b][mi][:, cs]
                    nc.vector.tensor_mul(dst, qraw[:], cos_sb[:, cs])
                    nc.vector.tensor_mul(qrot[:], qrot[:], sin_sb[:, cs])
                    nc.vector.tensor_add(dst, dst, qrot[:])

                def win0():
                    """window 0, kc-outer: streams against the split DMAs."""
                    xat = xacc(0)
                    cs = slice(0, TW)
                    pq = [psum.tile([128, TW], f32, tag="w", name="pq01")
                          for _ in range(2)]
                    pcd = psum.tile([128, TW2], f32, tag="py", bufs=1,
                                    name="pq23")
                    mi_ap = [pq[0][:], pq[1][:], pcd[:, 0:TW], pcd[:, TW:]]
                    for kc in range(NKC):
                        for mi in range(4):
                            nc.tensor.matmul(
                                mi_ap[mi], wqkat(kc, mi), xat(kc),
                                start=(kc == 0), stop=(kc == NKC - 1))
                        if kc == 4:
                            prefetch_x(1)
                    for mi in range(4):
                        rope(0, mi, mi_ap[mi], cs)
                    for tci in range(4):
                        pv = psum.tile([128, TW], f32, tag="w", name="pv")
                        for kc in range(NKC):
                            nc.tensor.matmul(
                                pv[:, 0:256],
                                xat(kc)[:, tci * 128:(tci + 1) * 128],
                                wv_sb[:, kc, :],
                                start=(kc == 0), stop=(kc == NKC - 1))
                        nc.vector.tensor_copy(v_sb[:, tci, :], pv[:, 0:256])

                def win_gen(tw):
                    """QKV projection + rope for one 512-wide t window.
                    8 yields: 4 q/k mi-streams, 4 v tci-streams."""
                    b, twb = divmod(tw, NTWB)
                    xat = xacc(tw)
                    cs = slice(twb * TW, (twb + 1) * TW)
                    for mi in range(4):
                        pqk = psum.tile([128, TW], f32, tag="w", name="pqk")
                        for kc in range(NKC):
                            nc.tensor.matmul(
                                pqk[:], wqkat(kc, mi), xat(kc),
                                start=(kc == 0), stop=(kc == NKC - 1))
                        rope(b, mi, pqk[:], cs)
                        if mi == 0 and tw + 1 < NW:
                            prefetch_x(tw + 1)
                        yield
                    for tci in range(4):
                        pv = psum.tile([128, TW], f32, tag="w", name="pv")
                        for kc in range(NKC):
                            nc.tensor.matmul(
                                pv[:, 0:256],
                                xat(kc)[:, tci * 128:(tci + 1) * 128],
                                wv_sb[:, kc, :],
                                start=(kc == 0), stop=(kc == NKC - 1))
                        nc.vector.tensor_copy(v_sb[:, tw * 4 + tci, :],
                                              pv[:, 0:256])
                        yield

                def attn_gen(b, hf, h):
                    """scoresT+softmax+attn@v for one (batch, tq-half, head).
                    17 yields: 16 pipelined tkc units + tail (A2A on h==1)."""
                    qh, kh = qk_sb[b][h], qk_sb[b][2 + h]
                    ssum = pB.tile([128, TW2], f16, tag="ssum", bufs=2,
                                   name="ssum")
                    py = psum.tile([128, TW2], f32, tag="py", bufs=1,
                                   name="py")
                    es = []

                    def attnv(i):
                        for j in range(2):
                            nc.tensor.matmul(
                                py[:, j * TW:(j + 1) * TW],
                                v_sb[:, b * NTC + i, h * 128:(h + 1) * 128],
                                es[i][:, j * TW:(j + 1) * TW],
                                start=(i == 0), stop=(i == NTC - 1))

                    for tkc in range(NTC):
                        sc = psum.tile([128, TW2], f32, tag="sc", name="sc")
                        for j in range(2):
                            tq0 = hf * TW2 + j * TW
                            nc.tensor.matmul(
                                sc[:, j * TW:(j + 1) * TW],
                                kh[:, tkc * 128:(tkc + 1) * 128],
                                qh[:, tq0:tq0 + TW],
                                start=True, stop=True)
                        e = pB.tile([128, TW2], f16, tag="e", bufs=4,
                                    name="e")
                        es.append(e)
                        nc.scalar.activation(e[:], sc[:], EXP, scale=SCALE)
                        if tkc == 0:
                            nc.vector.tensor_copy(ssum[:], e[:])
                        else:
                            nc.vector.tensor_add(ssum[:], ssum[:], e[:])
                        if tkc >= 2:
                            attnv(tkc - 2)
                        yield
                    attnv(NTC - 2)
                    attnv(NTC - 1)
                    # denominator: partition-reduce ssum via ones-matmuls into
                    # a rotating sc slot, then approx-reciprocal + broadcast
                    dn = psum.tile([128, TW2], f32, tag="sc", name="dn")
                    for j in range(2):
                        nc.tensor.matmul(dn[0:1, j * TW:(j + 1) * TW],
                                         ones16[:],
                                         ssum[:, j * TW:(j + 1) * TW],
                                         start=True, stop=True)
                    rc = pB.tile([1, TW2], f32, tag="rc", bufs=2, name="rc")
                    nc.vector.reciprocal_approx_fast(rc[:], dn[0:1, :])
                    rbs = pB.tile([128, TW2], f32, tag="rbs", bufs=2,
                                  name="rbs")
                    nc.gpsimd.partition_broadcast(rbs[:], rc[:])
                    ybf = pB.tile([128, TW2], f16, tag="ybf", bufs=2,
                                  name="ybf")
                    for j in range(2):
                        nc.vector.tensor_mul(ybf[:, j * TW:(j + 1) * TW],
                                             py[:, j * TW:(j + 1) * TW],
                                             rbs[:, j * TW:(j + 1) * TW])
                    # scatter into the A2A src layout [dst core, 256, 128];
                    # issued from ACT so the trigger isn't queued behind Sync
                    nc.scalar.dma_start(
                        y2[b][hf][:]
                        .rearrange("k (hh d) t -> hh d k t", hh=HL)[h],
                        ybf[:].rearrange("d (k t) -> d k t", k=NCORES))
                    if h == HL - 1:
                        nc.gpsimd.collective_compute(
                            "AllToAll",
                            mybir.AluOpType.bypass,
                            replica_groups=[list(range(NCORES))],
                            ins=[y2[b][hf][:]],
                            outs=[ya2[b][hf][:]],
                        )
                    yield

                def drive(gen, n=None):
                    if n is None:
                        for _ in gen:
                            pass
                    else:
                        for _ in range(n):
                            next(gen)

                # ---- front half ------------------------------------------
                win0()
                for twb in range(1, NTWB):
                    drive(win_gen(twb))
                for i in range(NTWB):
                    w = win_gen(NTWB + i)
                    a = attn_gen(0, i // 2, i % 2)
                    for _ in range(8):
                        drive(w, 1)
                        drive(a, 2)
                    drive(a)
                    # stage the Wproj lo-half load behind the startup traffic
                    if i in (0, 1):
                        nq = 4 * NKC          # 4 oc chunks per DMA
                        nc.sync.dma_start(
                            wp_lo[:, i * nq:(i + 1) * nq, :],
                            wpPC[:, i * nq * 128:(i + 1) * nq * 128]
                            .rearrange("p (c o) -> p c o", c=nq))

                pR_cm.__exit__(None, None, None)
                pA_cm.__exit__(None, None, None)

                # ---- back half -------------------------------------------
                with tc.tile_pool(name="pC", bufs=1) as pC:
                    wp_hi = pC.tile([128, 8 * NKC, 128], f16, tag="wphi")
                    HIO = 8 * NKC * 128

                    def dma_wp_hi(g):
                        # deferred so it doesn't starve the small y2 writes
                        nq = 4 * NKC
                        nc.sync.dma_start(
                            wp_hi[:, g * nq:(g + 1) * nq, :],
                            wpPC[:, HIO + g * nq * 128:
                                 HIO + (g + 1) * nq * 128]
                            .rearrange("p (c o) -> p c o", c=nq))

                    def wpat(oc, kc):
                        if oc < 8:
                            return wp_lo[:, oc * NKC + kc, :]
                        return wp_hi[:, (oc - 8) * NKC + kc, :]

                    yq = {}

                    def load_ya(b, hf):
                        t = pC.tile([128, NKC, TS], f16, tag="yq", bufs=2,
                                    name="yq")
                        nc.sync.dma_start(
                            t[:],
                            ya2[b][hf][:]
                            .rearrange("s (hh p) t -> p (s hh) t", p=128))
                        yq[(b, hf)] = t

                    def proj_gen(b, hf):
                        """all 2048 out chans for my 128 owned tokens of one
                        (b,hf) quarter; 16 yields (one per oc chunk)."""
                        yt = yq[(b, hf)]
                        q = 2 * b + hf
                        for oc in range(NKC):
                            po = psum.tile([128, TW], f32, tag="w",
                                           name="po")
                            for kc in range(NKC):
                                nc.tensor.matmul(
                                    po[:, 0:TS],
                                    wpat(oc, kc),
                                    yt[:, kc, :],
                                    start=(kc == 0), stop=(kc == NKC - 1))
                            od = pC.tile([128, TS], f16, tag="od", bufs=2,
                                         name="od")
                            if oc % 2 == 0:
                                nc.vector.tensor_copy(od[:], po[:, 0:TS])
                            else:
                                nc.scalar.copy(od[:], po[:, 0:TS])
                            nc.sync.dma_start(
                                outT[oc * 128:(oc + 1) * 128,
                                     q * TS:(q + 1) * TS],
                                od[:])
                            yield

                    # weave only 4 proj chunks per block slot; the ~35us of
                    # remaining collective-independent proj work then covers
                    # the last A2A's latency after the final block
                    load_ya(0, 0)
                    load_ya(0, 1)
                    weave = [(0, 0), (0, 0), (0, 1), (0, 1)]
                    pgen = {}
                    for i in range(4):
                        a = attn_gen(1, i // 2, i % 2)
                        key = weave[i]
                        if key not in pgen:
                            pgen[key] = proj_gen(*key)
                        p = pgen[key]
                        for u in range(NTC):
                            drive(a, 1)
                            if u % 4 == 3:
                                drive(p, 1)
                        drive(a)
                        if i == 1:
                            dma_wp_hi(0)
                        elif i == 2:
                            dma_wp_hi(1)
                    load_ya(1, 0)
                    drive(pgen[(0, 0)])
                    drive(pgen[(0, 1)])
                    load_ya(1, 1)
                    drive(proj_gen(1, 0))
                    drive(proj_gen(1, 1))
    nc.compile()
    return nc


def _prepare_in_maps(x, cos, sin, Wqkv, Wproj):
    f16 = np.float16

    def chunk(a):
        # [NKC*128, N] -> [128, NKC*N] per-partition-contiguous kc-major
        n = a.shape[1]
        return np.ascontiguousarray(
            a.reshape(NKC, 128, n).transpose(1, 0, 2).reshape(128, NKC * n))

    xT = x.reshape(TT, C).T.astype(f16)                      # [C, TT]
    xPC = np.empty((NW, 128, NKC * TW), dtype=f16)
    for tw in range(NW):
        xPC[tw] = chunk(xT[:, tw * TW:(tw + 1) * TW])
    cosPC = np.ascontiguousarray(cos.T).astype(f16)          # [128, T]
    sinS = sin.T.astype(np.float32).copy()
    sinS[:D // 2] *= -1.0
    sinPC = np.ascontiguousarray(sinS).astype(f16)
    Wq, Wk, Wv = Wqkv[0:C], Wqkv[C:2 * C], Wqkv[2 * C:3 * C]
    # Wproj oc-major: [p, (oc kc o)] with block(oc,kc) = WT[kc*128:, oc*128:]
    wpPC = np.ascontiguousarray(
        Wproj.T.astype(f16).reshape(NKC, 128, NKC, 128)
        .transpose(1, 2, 0, 3).reshape(128, NKC * NKC * 128))

    in_maps = []
    for c in range(NCORES):
        hs = [HL * c + j for j in range(HL)]
        wqk_rows = np.concatenate(
            [Wq[h * D:(h + 1) * D] for h in hs]
            + [Wk[h * D:(h + 1) * D] for h in hs], axis=0)
        wv_rows = np.concatenate([Wv[h * D:(h + 1) * D] for h in hs], axis=0)
        in_maps.append({
            "xPC": xPC,
            "wqkPC": chunk(wqk_rows.T.astype(f16)),
            "wvPC": chunk(wv_rows.T.astype(f16)),
            "wpPC": wpPC,
            "cosPC": cosPC,
            "sinPC": sinPC,
        })
    return in_maps


def run_sharded(x, cos, sin, Wqkv, Wproj, trace=False):
    """Compile (cached), run on 8 cores, return (out, BassKernelResults)."""
    from concourse.bass_utils import run_bass_kernel_spmd

    if "nc" not in _CACHE:
        _CACHE["nc"] = _build()
    nc = _CACHE["nc"]
    in_maps = _prepare_in_maps(np.asarray(x), np.asarray(cos),
                               np.asarray(sin), np.asarray(Wqkv),
                               np.asarray(Wproj))
    res = run_bass_kernel_spmd(nc, in_maps, core_ids=list(range(NCORES)),
                               trace=trace)
    out = np.empty((B, T, C), dtype=np.float32)
    for c in range(NCORES):
        # outT [2048, 4*128]: col q*128+tl is token (b, hf*1024+c*128+tl)
        oT = res.results[c]["outT"].astype(np.float32)
        oT = oT.reshape(C, 2, 2, TS)          # [oc, b, hf, tl]
        for b in range(B):
            for hf in range(2):
                t0 = hf * TW2 + c * TS
                out[b, t0:t0 + TS, :] = oT[:, b, hf, :].T
    return out, res


def kernel(x, cos, sin, Wqkv, Wproj):
    out, _ = run_sharded(x, cos, sin, Wqkv, Wproj, trace=False)
    return out


# revision 22
# speedup vs baseline: 1.3130x; 1.0421x over previous
"""Distributed Trainium2 kernel for a full attention block (QKV proj + RoPE +
bidirectional SDPA + output proj), SPMD across 8 NeuronCores.

Sharding: tensor-parallel over heads (16 heads -> 2 per core) for QKV+attention.
The merge for the output projection uses 4 AllToAlls (one per batch x tq-half
quarter): each core keeps a 128-token slice of every quarter and computes ALL
2048 output channels for its 512 owned tokens. A2A moves each y element once
(~0.45MB wire/core per quarter) vs AllGather's 8x (every element to every
core), which un-bottlenecks the serial CC stream that dominated the v2 tail.

v3 structure (v2 measured 509us, v1 539-584us; PE floor ~402us at the
observed ~0.5ns/col stream rate):
  - front half: 4 batch-0 QKV/RoPE windows (window 0 is kc-outer so the first
    matmul starts ~3us in, streaming against the split weight/x DMAs), then 4
    slots of batch-1 window || batch-0 attention block, with the block's 16
    (scores->exp->ssum->attn@v[skew-2]) units woven between the window's 8
    matmul streams so the in-order PE never waits on the Scalar-engine exp.
  - A2A(0,hf) fires mid-front; A2A(1,hf) fires as back-half blocks finish.
  - back half: 4 batch-1 blocks with batch-0 proj quarters woven between
    units; batch-1 proj runs last, its A2A wait hidden under the (1,0) proj.
  - q,k SBUF-resident; softmax denominator = f16 SBUF DVE accumulation +
    ones-matmul partition reduction + reciprocal_approx_fast + gpsimd
    broadcast.

dtypes: f16 everywhere on the matmul/exp/collective path, f32 psum, f16
output (host converts); full Wproj (8MB f16) is SBUF-resident per core.
"""
import sys
for _p in ("/opt/trn_rl_repo",):
    if _p not in sys.path:
        sys.path.append(_p)

import numpy as np

B, T, C = 2, 2048, 2048
H, D = 16, 128
NCORES = 8
HL = H // NCORES          # heads per core = 2
TT = B * T                # 4096
NKC = C // 128            # 16 contraction chunks
TW = 512                  # t-window (psum bank width in f32)
TW2 = 1024                # block tq width (2 banks)
NTWB = T // TW            # 4 x-windows per batch
NW = B * NTWB             # 8 windows total
NTC = T // 128            # 16 tk chunks per batch
TS = 128                  # per-core token slice per (b,hf) quarter
SCALE = float(1.0 / np.sqrt(D))

_CACHE = {}


def _build():
    from concourse import bacc, mybir, tile

    f32 = mybir.dt.float32
    f16 = mybir.dt.float16
    EXP = mybir.ActivationFunctionType.Exp

    nc = bacc.Bacc("TRN2", target_bir_lowering=False, debug=False,
                   num_devices=NCORES)

    # host pre-chunked layouts: per-partition-contiguous, no DMA rearranges
    xPC = nc.dram_tensor("xPC", [NW, 128, NKC * TW], f16, kind="ExternalInput")
    wqkPC = nc.dram_tensor("wqkPC", [128, NKC * 512], f16, kind="ExternalInput")
    wvPC = nc.dram_tensor("wvPC", [128, NKC * 256], f16, kind="ExternalInput")
    wpPC = nc.dram_tensor("wpPC", [128, NKC * 2048], f16,
                          kind="ExternalInput")
    cosPC = nc.dram_tensor("cosPC", [128, T], f16, kind="ExternalInput")
    sinPC = nc.dram_tensor("sinPC", [128, T], f16, kind="ExternalInput")
    # [2048 out chans, 4 quarters x 128 owned tokens]
    outT = nc.dram_tensor("outT", [C, 2 * 2 * TS], f16, kind="ExternalOutput")

    with tile.TileContext(nc) as tc:
        with tc.tile_pool(name="dram", bufs=1, space="DRAM") as dram:
            # A2A buffers per (batch, tq-half): in = [dst core, my 2 heads x
            # 128 d, 128 t], out = [src core (=head pair), 256, 128]
            y2 = [[dram.tile([NCORES, HL * 128, TS], f16, tag=f"y{b}{hf}",
                             name=f"y{b}{hf}") for hf in range(2)]
                  for b in range(B)]
            ya2 = [[dram.tile([NCORES, HL * 128, TS], f16, tag=f"ya{b}{hf}",
                              name=f"ya{b}{hf}")
                    for hf in range(2)] for b in range(B)]

            with (
                # PSUM tags (8 banks): sc [128,1024]x2 (scores + denom slots),
                # py [128,1024]x1 (attn@v accum / window-0 mi2+mi3),
                # w [128,512]x2 (qk-proj, v-proj, out-proj streams)
                tc.tile_pool(name="psum", bufs=2, space="PSUM") as psum,
                tc.tile_pool(name="pB", bufs=1) as pB,
            ):
                v_sb = pB.tile([128, B * NTC, HL * 128], f16, tag="v")
                # q_h0, q_h1, k_h0, k_h1 per batch, SBUF-resident
                qk_sb = [[pB.tile([128, T], f16, tag=f"qk{b}{mi}",
                                  name=f"qk{b}{mi}") for mi in range(4)]
                         for b in range(B)]
                ones16 = pB.tile([128, 1], f16, tag="ones16")
                nc.vector.memset(ones16[:], 1.0)
                # Wproj out-chans 0-1023, loaded mid-front (oc-major chunks);
                # the other half lives in the back-half pool
                wp_lo = pB.tile([128, 8 * NKC, 128], f16, tag="wplo")

                pA_cm = tc.tile_pool(name="pA", bufs=1)
                pA = pA_cm.__enter__()
                pR_cm = tc.tile_pool(name="pR", bufs=1)
                pR = pR_cm.__enter__()

                # ---- phase A prologue: critical-path-ordered split DMAs ----
                wqk8 = [pA.tile([128, 2, 512], f16, tag=f"wqk{g}",
                                name=f"wqk{g}") for g in range(8)]
                x0g = [pA.tile([128, 4, TW], f16, tag=f"x0{g}",
                               name=f"x0{g}") for g in range(4)]
                cos_sb = pA.tile([128, T], f16, tag="cos")
                sin_sb = pA.tile([128, T], f16, tag="sin")
                wv_sb = pA.tile([128, NKC, 256], f16, tag="wv")

                def dma_wqk(g):
                    nc.sync.dma_start(wqk8[g][:],
                                      wqkPC[:, g * 2 * 512:(g + 1) * 2 * 512]
                                      .rearrange("p (kc o) -> p kc o", kc=2))

                def dma_x0(g):
                    nc.sync.dma_start(x0g[g][:],
                                      xPC[0, :, g * 4 * TW:(g + 1) * 4 * TW]
                                      .rearrange("p (kc t) -> p kc t", kc=4))

                # interleaved by first-use order (kc-outer window 0)
                dma_wqk(0); dma_x0(0); dma_wqk(1); dma_x0(1)
                dma_wqk(2); dma_wqk(3); dma_x0(2)
                dma_wqk(4); dma_wqk(5); dma_x0(3)
                dma_wqk(6); dma_wqk(7)
                nc.sync.dma_start(wv_sb[:],
                                  wvPC[:].rearrange("p (kc o) -> p kc o",
                                                    kc=NKC))
                nc.sync.dma_start(cos_sb[:], cosPC[:])
                nc.sync.dma_start(sin_sb[:], sinPC[:])

                xtiles = {}

                def xacc(tw):
                    if tw == 0:
                        return lambda kc: x0g[kc // 4][:, kc % 4, :]
                    t = xtiles[tw]
                    return lambda kc: t[:, kc, :]

                def wqkat(kc, mi):
                    return wqk8[kc // 2][:, kc % 2, mi * 128:(mi + 1) * 128]

                def prefetch_x(tw):
                    t = pA.tile([128, NKC, TW], f16, tag="x", bufs=2,
                                name="x_sb")
                    nc.sync.dma_start(
                        t[:], xPC[tw].rearrange("p (kc t) -> p kc t", kc=NKC))
                    xtiles[tw] = t

                def rope(b, mi, src_ap, cs):
                    """q' = q*cos + swap_halves(q)*sin_signed, into qk_sb."""
                    qraw = pR.tile([128, TW], f16, tag="qraw", bufs=2,
                                   name="qraw")
                    nc.scalar.copy(qraw[:], src_ap)
                    qrot = pR.tile([128, TW], f16, tag="qrot", bufs=2,
                                   name="qrot")
                    nc.sync.dma_start(qrot[0:64, :], qraw[64:128, :])
                    nc.sync.dma_start(qrot[64:128, :], qraw[0:64, :])
                    dst = qk_sb[b][mi][:, cs]
                    nc.vector.tensor_mul(dst, qraw[:], cos_sb[:, cs])
                    nc.vector.tensor_mul(qrot[:], qrot[:], sin_sb[:, cs])
                    nc.vector.tensor_add(dst, dst, qrot[:])

                def win0():
                    """window 0, kc-outer: streams against the split DMAs."""
                    xat = xacc(0)
                    cs = slice(0, TW)
                    pq = [psum.tile([128, TW], f32, tag="w", name="pq01")
                          for _ in range(2)]
                    pcd = psum.tile([128, TW2], f32, tag="py", bufs=1,
                                    name="pq23")
                    mi_ap = [pq[0][:], pq[1][:], pcd[:, 0:TW], pcd[:, TW:]]
                    for kc in range(NKC):
                        for mi in range(4):
                            nc.tensor.matmul(
                                mi_ap[mi], wqkat(kc, mi), xat(kc),
                                start=(kc == 0), stop=(kc == NKC - 1))
                        if kc == 4:
                            prefetch_x(1)
                    for mi in range(4):
                        rope(0, mi, mi_ap[mi], cs)
                    for tci in range(4):
                        pv = psum.tile([128, TW], f32, tag="w", name="pv")
                        for kc in range(NKC):
                            nc.tensor.matmul(
                                pv[:, 0:256],
                                xat(kc)[:, tci * 128:(tci + 1) * 128],
                                wv_sb[:, kc, :],
                                start=(kc == 0), stop=(kc == NKC - 1))
                        nc.vector.tensor_copy(v_sb[:, tci, :], pv[:, 0:256])

                def win_gen(tw):
                    """QKV projection + rope for one 512-wide t window.
                    8 yields: 4 q/k mi-streams, 4 v tci-streams."""
                    b, twb = divmod(tw, NTWB)
                    xat = xacc(tw)
                    cs = slice(twb * TW, (twb + 1) * TW)
                    for mi in range(4):
                        pqk = psum.tile([128, TW], f32, tag="w", name="pqk")
                        for kc in range(NKC):
                            nc.tensor.matmul(
                                pqk[:], wqkat(kc, mi), xat(kc),
                                start=(kc == 0), stop=(kc == NKC - 1))
                        rope(b, mi, pqk[:], cs)
                        if mi == 0 and tw + 1 < NW:
                            prefetch_x(tw + 1)
                        yield
                    for tci in range(4):
                        pv = psum.tile([128, TW], f32, tag="w", name="pv")
                        for kc in range(NKC):
                            nc.tensor.matmul(
                                pv[:, 0:256],
                                xat(kc)[:, tci * 128:(tci + 1) * 128],
                                wv_sb[:, kc, :],
                                start=(kc == 0), stop=(kc == NKC - 1))
                        nc.vector.tensor_copy(v_sb[:, tw * 4 + tci, :],
                                              pv[:, 0:256])
                        yield

                def attn_gen(b, hf, h):
                    """scoresT+softmax+attn@v for one (batch, tq-half, head).
                    17 yields: 16 pipelined tkc units + tail (A2A on h==1)."""
                    qh, kh = qk_sb[b][h], qk_sb[b][2 + h]
                    ssum = pB.tile([128, TW2], f16, tag="ssum", bufs=2,
                                   name="ssum")
                    py = psum.tile([128, TW2], f32, tag="py", bufs=1,
                                   name="py")
                    es = []

                    def attnv(i):
                        for j in range(2):
                            nc.tensor.matmul(
                                py[:, j * TW:(j + 1) * TW],
                                v_sb[:, b * NTC + i, h * 128:(h + 1) * 128],
                                es[i][:, j * TW:(j + 1) * TW],
                                start=(i == 0), stop=(i == NTC - 1))

                    for tkc in range(NTC):
                        sc = psum.tile([128, TW2], f32, tag="sc", name="sc")
                        for j in range(2):
                            tq0 = hf * TW2 + j * TW
                            nc.tensor.matmul(
                                sc[:, j * TW:(j + 1) * TW],
                                kh[:, tkc * 128:(tkc + 1) * 128],
                                qh[:, tq0:tq0 + TW],
                                start=True, stop=True)
                        e = pB.tile([128, TW2], f16, tag="e", bufs=4,
                                    name="e")
                        es.append(e)
                        nc.scalar.activation(e[:], sc[:], EXP, scale=SCALE)
                        if tkc == 0:
                            nc.vector.tensor_copy(ssum[:], e[:])
                        else:
                            nc.vector.tensor_add(ssum[:], ssum[:], e[:])
                        if tkc >= 2:
                            attnv(tkc - 2)
                        yield
                    attnv(NTC - 2)
                    attnv(NTC - 1)
                    # denominator: partition-reduce ssum via ones-matmuls into
                    # a rotating sc slot, then approx-reciprocal + broadcast
                    dn = psum.tile([128, TW2], f32, tag="sc", name="dn")
                    for j in range(2):
                        nc.tensor.matmul(dn[0:1, j * TW:(j + 1) * TW],
                                         ones16[:],
                                         ssum[:, j * TW:(j + 1) * TW],
                                         start=True, stop=True)
                    rc = pB.tile([1, TW2], f32, tag="rc", bufs=2, name="rc")
                    nc.vector.reciprocal_approx_fast(rc[:], dn[0:1, :])
                    rbs = pB.tile([128, TW2], f32, tag="rbs", bufs=2,
                                  name="rbs")
                    nc.gpsimd.partition_broadcast(rbs[:], rc[:])
                    ybf = pB.tile([128, TW2], f16, tag="ybf", bufs=2,
                                  name="ybf")
                    for j in range(2):
                        nc.vector.tensor_mul(ybf[:, j * TW:(j + 1) * TW],
                                             py[:, j * TW:(j + 1) * TW],
                                             rbs[:, j * TW:(j + 1) * TW])
                    # scatter into the A2A src layout [dst core, 256, 128];
                    # issued from ACT so the trigger isn't queued behind Sync
                    nc.scalar.dma_start(
                        y2[b][hf][:]
                        .rearrange("k (hh d) t -> hh d k t", hh=HL)[h],
                        ybf[:].rearrange("d (k t) -> d k t", k=NCORES))
                    if h == HL - 1:
                        nc.gpsimd.collective_compute(
                            "AllToAll",
                            mybir.AluOpType.bypass,
                            replica_groups=[list(range(NCORES))],
                            ins=[y2[b][hf][:]],
                            outs=[ya2[b][hf][:]],
                        )
                    yield

                def drive(gen, n=None):
                    if n is None:
                        for _ in gen:
                            pass
                    else:
                        for _ in range(n):
                            next(gen)

                # ---- front half ------------------------------------------
                win0()
                for twb in range(1, NTWB):
                    drive(win_gen(twb))
                for i in range(NTWB):
                    w = win_gen(NTWB + i)
                    a = attn_gen(0, i // 2, i % 2)
                    for _ in range(8):
                        drive(w, 1)
                        drive(a, 2)
                    drive(a)
                    # stage the Wproj lo-half load behind the startup traffic
                    if i in (0, 1):
                        nq = 4 * NKC          # 4 oc chunks per DMA
                        nc.sync.dma_start(
                            wp_lo[:, i * nq:(i + 1) * nq, :],
                            wpPC[:, i * nq * 128:(i + 1) * nq * 128]
                            .rearrange("p (c o) -> p c o", c=nq))

                pR_cm.__exit__(None, None, None)
                pA_cm.__exit__(None, None, None)

                # ---- back half -------------------------------------------
                with tc.tile_pool(name="pC", bufs=1) as pC:
                    wp_hi = pC.tile([128, 8 * NKC, 128], f16, tag="wphi")
                    HIO = 8 * NKC * 128

                    def dma_wp_hi(g):
                        # deferred so it doesn't starve the small y2 writes
                        nq = 4 * NKC
                        nc.sync.dma_start(
                            wp_hi[:, g * nq:(g + 1) * nq, :],
                            wpPC[:, HIO + g * nq * 128:
                                 HIO + (g + 1) * nq * 128]
                            .rearrange("p (c o) -> p c o", c=nq))

                    def wpat(oc, kc):
                        if oc < 8:
                            return wp_lo[:, oc * NKC + kc, :]
                        return wp_hi[:, (oc - 8) * NKC + kc, :]

                    yq = {}

                    def load_ya(b, hf):
                        t = pC.tile([128, NKC, TS], f16, tag="yq", bufs=2,
                                    name="yq")
                        nc.sync.dma_start(
                            t[:],
                            ya2[b][hf][:]
                            .rearrange("s (hh p) t -> p (s hh) t", p=128))
                        yq[(b, hf)] = t

                    def proj_gen(b, hf):
                        """all 2048 out chans for my 128 owned tokens of one
                        (b,hf) quarter; 16 yields (one per oc chunk)."""
                        yt = yq[(b, hf)]
                        q = 2 * b + hf
                        od = None
                        for oc in range(NKC):
                            po = psum.tile([128, TW], f32, tag="w",
                                           name="po")
                            for kc in range(NKC):
                                nc.tensor.matmul(
                                    po[:, 0:TS],
                                    wpat(oc, kc),
                                    yt[:, kc, :],
                                    start=(kc == 0), stop=(kc == NKC - 1))
                            # pair oc chunks into one od tile / one out DMA
                            if oc % 2 == 0:
                                od = pC.tile([128, 2, TS], f16, tag="od",
                                             bufs=2, name="od")
                                nc.vector.tensor_copy(od[:, 0, :],
                                                      po[:, 0:TS])
                            else:
                                nc.scalar.copy(od[:, 1, :], po[:, 0:TS])
                                nc.sync.dma_start(
                                    outT[(oc - 1) * 128:(oc + 1) * 128,
                                         q * TS:(q + 1) * TS]
                                    .rearrange("(two p) t -> p two t", p=128),
                                    od[:])
                            yield

                    # slots weave only P(0,0) (its A2A lands early); all the
                    # bulk loads are emitted away from the block tails so the
                    # small y2 scatter-writes (which gate the A2A triggers)
                    # are never starved. ~60us of collective-independent proj
                    # work remains after the last block to cover A2A(1,1).
                    load_ya(0, 0)
                    p = proj_gen(0, 0)
                    for i in range(4):
                        a = attn_gen(1, i // 2, i % 2)
                        for u in range(NTC):
                            drive(a, 1)
                            if u % 4 == 3:
                                drive(p, 1)
                        drive(a)
                        if i == 0:
                            dma_wp_hi(0)
                        elif i == 1:
                            dma_wp_hi(1)
                            load_ya(0, 1)
                    drive(proj_gen(0, 1))
                    load_ya(1, 0)
                    drive(proj_gen(1, 0))
                    load_ya(1, 1)
                    drive(proj_gen(1, 1))
    nc.compile()
    return nc


def _prepare_in_maps(x, cos, sin, Wqkv, Wproj):
    f16 = np.float16

    def chunk(a):
        # [NKC*128, N] -> [128, NKC*N] per-partition-contiguous kc-major
        n = a.shape[1]
        return np.ascontiguousarray(
            a.reshape(NKC, 128, n).transpose(1, 0, 2).reshape(128, NKC * n))

    xT = x.reshape(TT, C).T.astype(f16)                      # [C, TT]
    xPC = np.empty((NW, 128, NKC * TW), dtype=f16)
    for tw in range(NW):
        xPC[tw] = chunk(xT[:, tw * TW:(tw + 1) * TW])
    cosPC = np.ascontiguousarray(cos.T).astype(f16)          # [128, T]
    sinS = sin.T.astype(np.float32).copy()
    sinS[:D // 2] *= -1.0
    sinPC = np.ascontiguousarray(sinS).astype(f16)
    Wq, Wk, Wv = Wqkv[0:C], Wqkv[C:2 * C], Wqkv[2 * C:3 * C]
    # Wproj oc-major: [p, (oc kc o)] with block(oc,kc) = WT[kc*128:, oc*128:]
    wpPC = np.ascontiguousarray(
        Wproj.T.astype(f16).reshape(NKC, 128, NKC, 128)
        .transpose(1, 2, 0, 3).reshape(128, NKC * NKC * 128))

    in_maps = []
    for c in range(NCORES):
        hs = [HL * c + j for j in range(HL)]
        wqk_rows = np.concatenate(
            [Wq[h * D:(h + 1) * D] for h in hs]
            + [Wk[h * D:(h + 1) * D] for h in hs], axis=0)
        wv_rows = np.concatenate([Wv[h * D:(h + 1) * D] for h in hs], axis=0)
        in_maps.append({
            "xPC": xPC,
            "wqkPC": chunk(wqk_rows.T.astype(f16)),
            "wvPC": chunk(wv_rows.T.astype(f16)),
            "wpPC": wpPC,
            "cosPC": cosPC,
            "sinPC": sinPC,
        })
    return in_maps


def run_sharded(x, cos, sin, Wqkv, Wproj, trace=False):
    """Compile (cached), run on 8 cores, return (out, BassKernelResults)."""
    from concourse.bass_utils import run_bass_kernel_spmd

    if "nc" not in _CACHE:
        _CACHE["nc"] = _build()
    nc = _CACHE["nc"]
    in_maps = _prepare_in_maps(np.asarray(x), np.asarray(cos),
                               np.asarray(sin), np.asarray(Wqkv),
                               np.asarray(Wproj))
    res = run_bass_kernel_spmd(nc, in_maps, core_ids=list(range(NCORES)),
                               trace=trace)
    out = np.empty((B, T, C), dtype=np.float32)
    for c in range(NCORES):
        # outT [2048, 4*128]: col q*128+tl is token (b, hf*1024+c*128+tl)
        oT = res.results[c]["outT"].astype(np.float32)
        oT = oT.reshape(C, 2, 2, TS)          # [oc, b, hf, tl]
        for b in range(B):
            for hf in range(2):
                t0 = hf * TW2 + c * TS
                out[b, t0:t0 + TS, :] = oT[:, b, hf, :].T
    return out, res


def kernel(x, cos, sin, Wqkv, Wproj):
    out, _ = run_sharded(x, cos, sin, Wqkv, Wproj, trace=False)
    return out
